# revision 98
# baseline (speedup 1.0000x reference)
"""Trainium2 Bass kernel: ComplexityAwareAttention (B=2, S=2048, D=1024, H=16).

Sharding: 8 cores = 2 batches x 4 head-groups (4 heads each).
Per core: QKV projections (bf16, head-slice, inputs pre-transposed on host),
flash-style causal attention with no-max softmax (scores bounded ~+-3.4),
complexity bias folded into V rows as exp(-cpen*c_k), partial O-projection,
bf16 ReduceScatter over the 4-core batch group, residual + LayerNorm on the
local quarter of rows.
"""

import numpy as np
import ml_dtypes
from contextlib import ExitStack

import concourse.bass as bass
import concourse.bacc as bacc
import concourse.tile as tile
from concourse import mybir
from concourse.bass_utils import run_bass_kernel_spmd


class _Exec:
    """Cached jit executor mirroring bass2jax.run_bass_via_pjrt (axon path),
    so repeat kernel() calls skip retracing and host->device re-staging of
    unchanged inputs can be controlled by the caller."""

    def __init__(self, nc, n_cores=8):
        import jax
        from jax.sharding import Mesh, PartitionSpec
        from jax.experimental.shard_map import shard_map
        from concourse import bass2jax
        from concourse import mybir as mb

        bass2jax.install_neuronx_cc_hook()
        assert nc.dbg_addr is None
        partition_name = (nc.partition_id_tensor.name
                          if nc.partition_id_tensor else None)
        in_names, out_names, out_avals = [], [], []
        for alloc in nc.m.functions[0].allocations:
            if not isinstance(alloc, mb.MemoryLocationSet):
                continue
            name = alloc.memorylocations[0].name
            if alloc.kind == "ExternalInput":
                if name != partition_name:
                    in_names.append(name)
            elif alloc.kind == "ExternalOutput":
                shape = tuple(alloc.tensor_shape)
                dtype = mb.dt.np(alloc.dtype)
                out_names.append(name)
                out_avals.append(jax.core.ShapedArray(shape, dtype))
        self.nc = nc
        self.in_names = in_names
        self.out_names = out_names
        self.out_avals = out_avals
        self.n_cores = n_cores
        n_params = len(in_names)
        n_outs = len(out_names)
        donate = tuple(range(n_params, n_params + n_outs))
        all_names = in_names + out_names
        if partition_name is not None:
            all_names = all_names + [partition_name]

        def _body(*args):
            operands = list(args)
            if partition_name is not None:
                operands.append(bass2jax.partition_id_tensor())
            return tuple(bass2jax._bass_exec_p.bind(
                *operands,
                out_avals=tuple(out_avals),
                in_names=tuple(all_names),
                out_names=tuple(out_names),
                lowering_input_output_aliases=(),
                sim_require_finite=True,
                sim_require_nnan=True,
                nc=nc,
            ))

        devices = jax.devices()[:n_cores]
        self.mesh = Mesh(np.asarray(devices), ("core",))
        in_specs = (PartitionSpec("core"),) * (n_params + n_outs)
        out_specs = (PartitionSpec("core"),) * n_outs
        self.sharded = jax.jit(
            shard_map(_body, mesh=self.mesh, in_specs=in_specs,
                      out_specs=out_specs, check_rep=False),
            donate_argnums=donate, keep_unused=True)
        self._jax = jax

    def stage(self, in_maps):
        """Concatenate per-core inputs and move to devices; returns arg list."""
        import jax
        from jax.sharding import NamedSharding, PartitionSpec
        sh = NamedSharding(self.mesh, PartitionSpec("core"))
        args = []
        for name in self.in_names:
            cat = np.concatenate([np.asarray(m[name]) for m in in_maps], axis=0)
            args.append(jax.device_put(cat, sh))
        return args

    def zeros(self):
        import jax
        from jax.sharding import NamedSharding, PartitionSpec
        sh = NamedSharding(self.mesh, PartitionSpec("core"))
        return [jax.device_put(
                    np.zeros((self.n_cores * a.shape[0], *a.shape[1:]), a.dtype), sh)
                for a in self.out_avals]

    def run(self, staged_args, zeros=None):
        if zeros is None:
            zeros = self.zeros()
        outs = self.sharded(*staged_args, *zeros)
        self._jax.block_until_ready(outs)
        return outs

    def make_chain(self, n):
        """Jitted fn running the program n times back-to-back on device;
        outputs of call i feed the (ignored, fully-overwritten) output
        buffers of call i+1, forcing serial execution."""
        import jax
        from jax.sharding import PartitionSpec
        from jax.experimental.shard_map import shard_map
        from concourse import bass2jax

        partition_name = (self.nc.partition_id_tensor.name
                          if self.nc.partition_id_tensor else None)
        all_names = self.in_names + self.out_names
        if partition_name is not None:
            all_names = all_names + [partition_name]
        out_avals = self.out_avals
        nc = self.nc

        def _body(*args):
            n_params = len(self.in_names)
            ins = list(args[:n_params])
            z = list(args[n_params:])
            for _ in range(n):
                operands = ins + z
                if partition_name is not None:
                    operands.append(bass2jax.partition_id_tensor())
                outs = bass2jax._bass_exec_p.bind(
                    *operands,
                    out_avals=tuple(out_avals),
                    in_names=tuple(all_names),
                    out_names=tuple(self.out_names),
                    lowering_input_output_aliases=(),
                    sim_require_finite=True,
                    sim_require_nnan=True,
                    nc=nc,
                )
                z = list(outs)
            return tuple(z)

        n_io = len(self.in_names) + len(self.out_names)
        in_specs = (PartitionSpec("core"),) * n_io
        out_specs = (PartitionSpec("core"),) * len(self.out_names)
        return jax.jit(
            shard_map(_body, mesh=self.mesh, in_specs=in_specs,
                      out_specs=out_specs, check_rep=False),
            keep_unused=True)

    def results(self, outs):
        per_core = []
        for c in range(self.n_cores):
            d = {}
            for i, name in enumerate(self.out_names):
                a = self.out_avals[i]
                d[name] = np.asarray(outs[i]).reshape(
                    self.n_cores, *a.shape)[c]
            per_core.append(d)
        return per_core

# ---- problem constants (hardcoded per harness contract)
B, S, D, H = 2, 2048, 1024, 16
DK = D // H                      # 64
EPS = 1e-5
P = 128
HL = 4                           # heads per core
HD = HL * DK                     # 256 local head dims
SQ = S // 4                      # 512 output rows per core
NDT = D // P                     # 8 d-tiles
NQT = 4                          # q tiles of 512
QW = 512
NST = S // P                     # 16 s/k tiles
GROUPS = [[0, 1, 2, 3], [4, 5, 6, 7]]

f32 = mybir.dt.float32
bf16 = mybir.dt.bfloat16
f32r = mybir.dt.float32r
BF = ml_dtypes.bfloat16

Alu = mybir.AluOpType
Act = mybir.ActivationFunctionType

_BUILD_CACHE = {}

_ET_BUFS = 8  # exp-tile ring depth (absorbs ACT-vs-PE skew)


def _build(n_rep=1):
    key = ("nc", n_rep, _ET_BUFS)
    if key in _BUILD_CACHE:
        return _BUILD_CACHE[key]

    nc = bacc.Bacc("TRN2", target_bir_lowering=False, debug=False,
                   enable_asserts=False, num_devices=8)

    # ---- DRAM I/O (x inputs pre-transposed on host: [D, S])
    xqT_t = nc.dram_tensor("xqT", [D, S], bf16, kind="ExternalInput")
    xkT_t = nc.dram_tensor("xkT", [D, S], bf16, kind="ExternalInput")
    xvT_t = nc.dram_tensor("xvT", [D, S], bf16, kind="ExternalInput")
    wqT = nc.dram_tensor("wqT", [D, HD], bf16, kind="ExternalInput").ap()
    wkT = nc.dram_tensor("wkT", [D, HD], bf16, kind="ExternalInput").ap()
    wvT = nc.dram_tensor("wvT", [D, HD], bf16, kind="ExternalInput").ap()
    woT = nc.dram_tensor("woT", [HD, D], bf16, kind="ExternalInput").ap()
    bqk = nc.dram_tensor("bqk", [P, 4], f32, kind="ExternalInput").ap()
    bvm = nc.dram_tensor("bvm", [1, HD], bf16, kind="ExternalInput").ap()
    enb = nc.dram_tensor("enb", [P, NST], f32, kind="ExternalInput").ap()
    masks2 = nc.dram_tensor("masks2", [4, P, 2 * QW], bf16, kind="ExternalInput").ap()
    selp = nc.dram_tensor("selp", [HL, 2 * P], f32r, kind="ExternalInput").ap()
    xres = nc.dram_tensor("xres", [SQ, D], f32, kind="ExternalInput").ap()
    lngb = nc.dram_tensor("lngb", [P, D], f32, kind="ExternalInput").ap()
    lnbb = nc.dram_tensor("lnbb", [P, D], f32, kind="ExternalInput").ap()
    y = nc.dram_tensor("y", [SQ, D], f32, kind="ExternalOutput").ap()

    rs_in = nc.dram_tensor("rs_in", [S, D], bf16).ap()
    rs_out = nc.dram_tensor("rs_out", [SQ, D], bf16).ap()

    with tile.TileContext(nc) as tc, ExitStack() as ctx:
        consts = ctx.enter_context(tc.tile_pool(name="consts", bufs=1))
        xt_pool = ctx.enter_context(tc.tile_pool(name="xt", bufs=1))
        et_pool = ctx.enter_context(tc.tile_pool(name="et", bufs=_ET_BUFS))
        stg_pool = ctx.enter_context(tc.tile_pool(name="stg", bufs=2))
        out_pool = ctx.enter_context(tc.tile_pool(name="outp", bufs=3))
        ln_pool = ctx.enter_context(tc.tile_pool(name="ln", bufs=2))
        pp_ps = ctx.enter_context(tc.tile_pool(name="pp_ps", bufs=2, space="PSUM"))
        s_ps = ctx.enter_context(tc.tile_pool(name="s_ps", bufs=2, space="PSUM"))
        c_ps = ctx.enter_context(tc.tile_pool(name="c_ps", bufs=1, space="PSUM"))

        # ---- true constants (outside the timing loop)
        ones_sb = consts.tile([1, P], bf16, name="ones_sb")
        nc.vector.memset(ones_sb, 1.0)
        eps_sb = consts.tile([P, 1], f32, name="eps_sb")
        nc.vector.memset(eps_sb, EPS)
        mask_sb = []
        for j in range(4):
            t = consts.tile([P, 2 * QW], bf16, name=f"mask{j}")
            nc.sync.dma_start(out=t, in_=masks2[j])
            mask_sb.append(t)
        selp_sb = consts.tile([HL, 2 * P], f32r, name="selp_sb")
        nc.sync.dma_start(out=selp_sb, in_=selp)

        # persistent activation tiles
        QT_sb = [consts.tile([P, S], bf16, name=f"QT{g}") for g in range(2)]
        KT_sb = [consts.tile([P, S], bf16, name=f"KT{g}") for g in range(2)]
        # V tiles: [V(64), enb] per head
        V_sb = [consts.tile([P, HL, DK + 1], bf16, name=f"V{st}") for st in range(NST)]
        cpair = [consts.tile([P, S], f32, name=f"cp{pr}") for pr in range(2)]
        ctxN = [consts.tile([P, S], bf16, name=f"cn{pr}") for pr in range(2)]
        rsums = consts.tile([HL, S], f32, name="rsums")
        rsr = consts.tile([HL, S], f32r, name="rsr")
        # weight tiles (persistent slots; reloaded per iteration in-loop)
        wq_sb = [consts.tile([P, HD], bf16, name=f"wq{d}") for d in range(NDT)]
        wk_sb = [consts.tile([P, HD], bf16, name=f"wk{d}") for d in range(NDT)]
        wv_sb = [consts.tile([P, HD], bf16, name=f"wv{d}") for d in range(NDT)]
        wo_sb = [consts.tile([P, D], bf16, name=f"wo{j}") for j in range(2)]
        bqk_sb = consts.tile([P, 4], f32, name="bqk_sb")
        bvm_sb = consts.tile([1, HD], bf16, name="bvm_sb")
        enb_sb = consts.tile([P, NST], f32, name="enb_sb")

        def load_x(dst, src_t, nchunk, eng):
            """DMA [D, S] dram -> [128, NDT*S] sbuf in nchunk pieces."""
            nd = NDT // nchunk
            for c in range(nchunk):
                d0 = c * nd
                dst3 = dst[:, d0 * S:(d0 + nd) * S].rearrange(
                    "p (d s) -> p d s", d=nd)
                src3 = bass.AP(tensor=src_t, offset=d0 * P * S,
                               ap=[[S, P], [P * S, nd], [1, S]])
                eng.dma_start(out=dst3, in_=src3)

        def emit_body():
            # ---- input/weight loads.
            # scalar (ACT) HWDGE queue: wq, bqk, wk, xkT, wv, bvm, enb
            #   (everything gating QK proj + V proj; done before exp stream)
            # sync (SP) HWDGE queue: xqT, xvT, wo (then rsum/rs_in writes)
            for d in range(NDT):
                nc.scalar.dma_start(out=wq_sb[d], in_=wqT[d * P:(d + 1) * P, :])
            nc.scalar.dma_start(out=bqk_sb, in_=bqk)
            for d in range(NDT):
                nc.scalar.dma_start(out=wk_sb[d], in_=wkT[d * P:(d + 1) * P, :])

            xq_sb = xt_pool.tile([P, NDT * S], bf16, name="xq_sb", tag="xq")
            load_x(xq_sb, xqT_t, 4, nc.sync)
            xk_sb = xt_pool.tile([P, NDT * S], bf16, name="xk_sb", tag="xk")
            load_x(xk_sb, xkT_t, 2, nc.scalar)

            for d in range(NDT):
                nc.scalar.dma_start(out=wv_sb[d], in_=wvT[d * P:(d + 1) * P, :])
            nc.scalar.dma_start(out=bvm_sb, in_=bvm)
            nc.scalar.dma_start(out=enb_sb, in_=enb)

            xv_sb = xt_pool.tile([P, NDT * S], bf16, name="xv_sb", tag="xq")
            load_x(xv_sb, xvT_t, 2, nc.sync)
            for j in range(2):
                nc.sync.dma_start(out=wo_sb[j], in_=woT[j * P:(j + 1) * P, :])

            # ---- Q/K projections -> QT/KT [2][128(2 heads x 64), S]
            def qk_proj(x_sb, w_sb, bbase, out_sb):
                for g in range(2):
                    for q in range(NQT):
                        ps = pp_ps.tile([P, QW], f32, name="pp", tag="pp")
                        for d in range(NDT):
                            nc.tensor.matmul(
                                ps, lhsT=w_sb[d][:, g * P:(g + 1) * P],
                                rhs=x_sb[:, d * S + q * QW:d * S + (q + 1) * QW],
                                start=(d == 0), stop=(d == NDT - 1))
                        nc.vector.tensor_scalar(
                            out=out_sb[g][:, q * QW:(q + 1) * QW], in0=ps,
                            scalar1=bqk_sb[:, bbase + g:bbase + g + 1],
                            scalar2=None, op0=Alu.add)

            qk_proj(xq_sb, wq_sb, 0, QT_sb)

            # ---- V projection
            def v_proj():
                for st in range(NST):
                    ps = pp_ps.tile([P, QW], f32, name="ppv", tag="pp")
                    psv = ps[:, 0:HD]
                    nc.tensor.matmul(psv, lhsT=ones_sb[0:1, 0:P], rhs=bvm_sb,
                                     start=True, stop=False)
                    for d in range(NDT):
                        nc.tensor.matmul(
                            psv, lhsT=xv_sb[:, d * S + st * P:d * S + (st + 1) * P],
                            rhs=wv_sb[d],
                            start=False, stop=(d == NDT - 1))
                    psr = psv.rearrange("p (h c) -> p h c", h=HL)
                    nc.vector.tensor_scalar(
                        out=V_sb[st][:, :, 0:DK], in0=psr,
                        scalar1=enb_sb[:, st:st + 1], scalar2=None, op0=Alu.mult)
                    nc.vector.tensor_copy(
                        out=V_sb[st][:, :, DK:DK + 1],
                        in_=enb_sb[:, st:st + 1].to_broadcast([P, HL, 1]))

            qk_proj(xk_sb, wk_sb, 2, KT_sb)
            v_proj()

            # ---- attention (head pairs; scores packed into array quadrants)
            for pr in range(2):
                g = pr
                for q in range(NQT):
                    nkt = 4 * (q + 1)
                    qs = slice(q * QW, (q + 1) * QW)
                    ctxA_t = c_ps.tile([P, QW], f32, name="ctxA", tag="cpA")
                    ctxB_t = c_ps.tile([P, QW], f32, name="ctxB", tag="cpB")
                    ctxA = ctxA_t[0:DK + 1, :]
                    ets = []
                    for kt in range(nkt):
                        j = kt - 4 * q  # >=0 on diagonal group
                        rs0 = 128 * j if j >= 0 else 0
                        # scoresT for both heads of the pair in one pass:
                        # 4 concurrent [K=64, M=64, N<=512] quadrant matmuls.
                        ps = s_ps.tile([P, 2 * QW], f32, name="sc", tag="sp")
                        for hh in range(2):
                            ho = hh * 64
                            for jj in range(2):
                                nc.tensor.matmul(
                                    ps[jj * 64:(jj + 1) * 64,
                                       hh * QW + rs0:(hh + 1) * QW],
                                    lhsT=KT_sb[g][ho:ho + 64,
                                                  kt * P + jj * 64:
                                                  kt * P + (jj + 1) * 64],
                                    rhs=QT_sb[g][ho:ho + 64,
                                                 q * QW + rs0:(q + 1) * QW],
                                    start=True, stop=True,
                                    tile_position=(ho, jj * 64))
                        et = et_pool.tile([P, 2, QW], bf16, name="et", tag="et")
                        ps3 = ps.rearrange("p (h f) -> p h f", h=2)
                        nc.scalar.activation(out=et[:, :, rs0:QW],
                                             in_=ps3[:, :, rs0:QW],
                                             func=Act.Exp, scale=0.125)
                        if j >= 0:
                            nc.vector.tensor_mul(
                                et[:, :, rs0:QW],
                                et[:, :, rs0:QW],
                                mask_sb[j].rearrange(
                                    "p (h f) -> p h f", h=2)[:, :, rs0:QW])
                        ets.append((et, rs0))
                    for kt, (et, rs0) in enumerate(ets):
                        nc.tensor.matmul(
                            ctxA[:, rs0:QW], lhsT=V_sb[kt][:, 2 * pr, :],
                            rhs=et[:, 0, rs0:QW],
                            start=(kt == 0), stop=(kt == nkt - 1))
                        # odd head: V rows -> partitions 64:128 (col groups
                        # 2-3); its enb sum row -> partition 32 (col group 1,
                        # runs concurrent with the V matmul)
                        nc.tensor.matmul(
                            ctxB_t[64:P, rs0:QW],
                            lhsT=V_sb[kt][:, 2 * pr + 1, 0:DK],
                            rhs=et[:, 1, rs0:QW],
                            start=(kt == 0), stop=(kt == nkt - 1),
                            tile_position=(0, 64))
                        nc.tensor.matmul(
                            ctxB_t[32:33, rs0:QW],
                            lhsT=V_sb[kt][:, 2 * pr + 1, DK:DK + 1],
                            rhs=et[:, 1, rs0:QW],
                            start=(kt == 0), stop=(kt == nkt - 1),
                            tile_position=(0, 32))
                    # ctxA rows: V@0:64, sum@64 ; ctxB rows: sum@32, V@64:128
                    nc.vector.tensor_copy(out=cpair[pr][0:DK, qs],
                                          in_=ctxA[0:DK, :])
                    nc.vector.tensor_copy(out=cpair[pr][64:P, qs],
                                          in_=ctxB_t[64:P, :])
                    srow = stg_pool.tile([P, QW], f32, name="srow", tag="sr")
                    nc.vector.tensor_copy(out=srow[64:65, :],
                                          in_=ctxA_t[DK:DK + 1, :])
                    nc.vector.tensor_copy(out=srow[32:33, :],
                                          in_=ctxB_t[32:33, :])
                    nc.sync.dma_start(out=rsums[2 * pr:2 * pr + 1, qs],
                                      in_=srow[64:65, :])
                    nc.sync.dma_start(out=rsums[2 * pr + 1:2 * pr + 2, qs],
                                      in_=srow[32:33, :])

            # ---- normalize ctx -> ctxN (bf16)
            nc.vector.reciprocal(out=rsums, in_=rsums)
            nc.vector.tensor_copy(out=rsr, in_=rsums)
            for pr in range(2):
                for q in range(NQT):
                    qs = slice(q * QW, (q + 1) * QW)
                    psb = pp_ps.tile([P, QW], f32, name="bcps", tag="pp")
                    nc.tensor.matmul(
                        psb,
                        lhsT=selp_sb[:, pr * P:(pr + 1) * P],
                        rhs=rsr[:, qs],
                        start=True, stop=True)
                    nc.vector.tensor_mul(ctxN[pr][:, qs], cpair[pr][:, qs], psb)

            # ---- O projection -> rs_in (bf16)
            for qb in range(NST):
                for dh in range(2):
                    ps = pp_ps.tile([P, QW], f32, name="ops", tag="pp")
                    for pr in range(2):
                        nc.tensor.matmul(
                            ps, lhsT=ctxN[pr][:, qb * P:(qb + 1) * P],
                            rhs=wo_sb[pr][:, dh * QW:(dh + 1) * QW],
                            start=(pr == 0), stop=(pr == 1))
                    o_sb = out_pool.tile([P, QW], bf16, name="o_sb", tag="ob")
                    if qb % 2 == 0:
                        nc.vector.tensor_copy(out=o_sb, in_=ps)
                    else:
                        nc.scalar.copy(out=o_sb, in_=ps)
                    nc.sync.dma_start(
                        out=rs_in[qb * P:(qb + 1) * P, dh * QW:(dh + 1) * QW],
                        in_=o_sb)

        def emit_finish():
            # ---- ReduceScatter over the batch group (bf16)
            nc.gpsimd.collective_compute(
                "ReduceScatter", Alu.add, replica_groups=GROUPS,
                ins=[rs_in.opt()], outs=[rs_out.opt()])

            lng_sb = consts.tile([P, D], f32, name="lng_sb")
            nc.scalar.dma_start(out=lng_sb, in_=lngb)
            lnb_sb = consts.tile([P, D], f32, name="lnb_sb")
            nc.scalar.dma_start(out=lnb_sb, in_=lnbb)

            # ---- residual + LayerNorm on local rows
            for t in range(SQ // P):
                rsl = slice(t * P, (t + 1) * P)
                rs_sb = ln_pool.tile([P, D], bf16, name="rs_sb", tag="lrs")
                nc.sync.dma_start(out=rs_sb, in_=rs_out[rsl, :])
                xr_sb = ln_pool.tile([P, D], f32, name="xr_sb", tag="lr")
                nc.sync.dma_start(out=xr_sb, in_=xres[rsl, :])
                x_sb = ln_pool.tile([P, D], f32, name="x_sb", tag="lx")
                nc.vector.tensor_copy(out=x_sb, in_=rs_sb)
                nc.vector.tensor_add(x_sb, x_sb, xr_sb)
                stats = ln_pool.tile([P, 2, 6], f32, name="stats", tag="lst")
                for sg in range(2):
                    nc.vector.bn_stats(out=stats[:, sg, :],
                                       in_=x_sb[:, sg * QW:(sg + 1) * QW])
                mv = ln_pool.tile([P, 2], f32, name="mv", tag="lmv")
                nc.vector.bn_aggr(out=mv, in_=stats)
                nc.scalar.activation(out=mv[:, 1:2], in_=mv[:, 1:2],
                                     func=Act.Sqrt, bias=eps_sb, scale=1.0)
                nc.vector.reciprocal(out=mv[:, 1:2], in_=mv[:, 1:2])
                nc.vector.tensor_scalar(
                    out=x_sb, in0=x_sb, scalar1=mv[:, 0:1], scalar2=mv[:, 1:2],
                    op0=Alu.subtract, op1=Alu.mult)
                nc.vector.tensor_mul(x_sb, x_sb, lng_sb)
                nc.vector.tensor_add(x_sb, x_sb, lnb_sb)
                nc.sync.dma_start(out=y[rsl, :], in_=x_sb)

        if n_rep == 1:
            emit_body()
        else:
            with tc.For_i(0, n_rep, 1):
                emit_body()
        emit_finish()

    nc.compile()
    _BUILD_CACHE[key] = nc
    return nc


def _make_masks():
    # mask[j][p, half*512 + f] = 1.0 if p + j*128 <= f else 0 (same both halves;
    # halves hold the two heads of a pair for the same k-tile)
    m = np.zeros((4, P, 2 * QW), dtype=np.float32)
    p = np.arange(P)[:, None]
    f = np.arange(QW)[None, :]
    for j in range(4):
        o = j * P
        keep = (p + o <= f)
        m[j][:, 0:QW] = keep
        m[j][:, QW:2 * QW] = keep
    return m.astype(BF)


def _make_selp():
    sp = np.zeros((HL, 2 * P), dtype=np.float32)
    mm = np.arange(P)
    for pr in range(2):
        for k in range(HL):
            sp[k, pr * P:(pr + 1) * P] = (k == 2 * pr + mm // 64)
    return sp


def _prep_in_maps(query, key, value, complexity, wq, bq, wk, bk, wv, bv,
                  wo, bo, ln_g, ln_b, cpen):
    masks2 = _make_masks()
    selp = _make_selp()
    lngb = np.ascontiguousarray(
        np.broadcast_to(np.asarray(ln_g, np.float32)[None, :], (P, D)))
    lnbb = np.ascontiguousarray(
        np.broadcast_to(np.asarray(ln_b, np.float32)[None, :], (P, D)))
    per_batch = []
    for b in range(B):
        xqT = np.ascontiguousarray(query[b].astype(BF).T)
        xkT = np.ascontiguousarray(key[b].astype(BF).T)
        xvT = np.ascontiguousarray(value[b].astype(BF).T)
        e = np.exp(-float(cpen) * complexity[b].astype(np.float64)).astype(np.float32)
        enb_l = np.ascontiguousarray(e.reshape(NST, P).T)
        per_batch.append((xqT, xkT, xvT, enb_l))
    in_maps = []
    for c in range(8):
        b, r = c // 4, c % 4
        hs = HD * r
        xqT, xkT, xvT, enb_l = per_batch[b]
        bqc = bq[hs:hs + HD].astype(np.float32).reshape(2, P).T  # [P, 2]
        bkc = bk[hs:hs + HD].astype(np.float32).reshape(2, P).T
        bqk_l = np.ascontiguousarray(
            np.concatenate([bqc, bkc], axis=1))  # [P,4]: q_g0,q_g1,k_g0,k_g1
        in_maps.append({
            "xqT": xqT, "xkT": xkT, "xvT": xvT,
            "wqT": np.ascontiguousarray(wq[hs:hs + HD, :].T).astype(BF),
            "wkT": np.ascontiguousarray(wk[hs:hs + HD, :].T).astype(BF),
            "wvT": np.ascontiguousarray(wv[hs:hs + HD, :].T).astype(BF),
            "woT": np.ascontiguousarray(wo[:, hs:hs + HD].T).astype(BF),
            "bqk": bqk_l,
            "bvm": bv[hs:hs + HD].astype(BF)[None, :],
            "enb": enb_l,
            "masks2": masks2,
            "selp": selp,
            "xres": (query[b][SQ * r:SQ * (r + 1)].astype(np.float32)
                     + bo.astype(np.float32)[None, :]),
            "lngb": lngb, "lnbb": lnbb,
        })
    return in_maps


def _numpy_fallback(query, key, value, complexity, mask, wq, bq, wk, bk,
                    wv, bv, wo, bo, ln_g, ln_b, cpen):
    import math
    out = np.zeros((B, S, D), np.float32)
    for b in range(B):
        Q = query[b] @ wq.T + bq
        K = key[b] @ wk.T + bk
        V = value[b] @ wv.T + bv
        Qh = Q.reshape(S, H, DK).transpose(1, 0, 2)
        Kh = K.reshape(S, H, DK).transpose(1, 0, 2)
        Vh = V.reshape(S, H, DK).transpose(1, 0, 2)
        ctx = np.zeros((H, S, DK), np.float32)
        m = mask[b, 0]
        for h in range(H):
            sc = Qh[h] @ Kh[h].T / math.sqrt(DK) - cpen * complexity[b][None, :]
            sc = np.where(m, sc, -1e9)
            sc = sc - sc.max(-1, keepdims=True)
            e = np.exp(sc)
            a = e / e.sum(-1, keepdims=True)
            ctx[h] = a @ Vh[h]
        x = ctx.transpose(1, 0, 2).reshape(S, D) @ wo.T + bo + query[b]
        mu = x.mean(-1, keepdims=True)
        var = ((x - mu) ** 2).mean(-1, keepdims=True)
        out[b] = (x - mu) / np.sqrt(var + EPS) * ln_g + ln_b
    return out


_TRIL = None


def kernel(query, key, value, complexity, mask, wq, bq, wk, bk, wv, bv,
           wo, bo, ln_g, ln_b, cpen, **_unused):
    query = np.asarray(query, dtype=np.float32)
    key = np.asarray(key, dtype=np.float32)
    value = np.asarray(value, dtype=np.float32)
    complexity = np.asarray(complexity, dtype=np.float32)
    mask = np.asarray(mask)
    args = dict(query=query, key=key, value=value, complexity=complexity,
                wq=np.asarray(wq), bq=np.asarray(bq), wk=np.asarray(wk),
                bk=np.asarray(bk), wv=np.asarray(wv), bv=np.asarray(bv),
                wo=np.asarray(wo), bo=np.asarray(bo),
                ln_g=np.asarray(ln_g), ln_b=np.asarray(ln_b),
                cpen=float(np.asarray(cpen)))
    global _TRIL
    if _TRIL is None:
        _TRIL = np.tril(np.ones((S, S), bool))
    if not all(np.array_equal(mask[b, 0], _TRIL) for b in range(B)):
        # non-causal mask: fall back to a generic host implementation
        return _numpy_fallback(mask=mask, **args)

    ex = _get_exec()
    in_maps = _prep_in_maps(**args)
    outs = ex.run(ex.stage(in_maps))
    res = ex.results(outs)
    out = np.empty((B, S, D), np.float32)
    for c in range(8):
        b, r = c // 4, c % 4
        out[b, SQ * r:SQ * (r + 1)] = res[c]["y"]
    return out


def _get_exec():
    if "ex" not in _BUILD_CACHE:
        _BUILD_CACHE["ex"] = _Exec(_build())
    return _BUILD_CACHE["ex"]


def _input_args(inputs):
    return dict(query=np.asarray(inputs["query"], np.float32),
                key=np.asarray(inputs["key"], np.float32),
                value=np.asarray(inputs["value"], np.float32),
                complexity=np.asarray(inputs["complexity"], np.float32),
                wq=np.asarray(inputs["wq"]), bq=np.asarray(inputs["bq"]),
                wk=np.asarray(inputs["wk"]), bk=np.asarray(inputs["bk"]),
                wv=np.asarray(inputs["wv"]), bv=np.asarray(inputs["bv"]),
                wo=np.asarray(inputs["wo"]), bo=np.asarray(inputs["bo"]),
                ln_g=np.asarray(inputs["ln_g"]), ln_b=np.asarray(inputs["ln_b"]),
                cpen=float(np.asarray(inputs["cpen"])))


# revision 104
# speedup vs baseline: 1.2182x; 1.2182x over previous
"""Trainium2 Bass kernel: ComplexityAwareAttention (B=2, S=2048, D=1024, H=16).

Sharding: 8 cores = 2 batches x 4 head-groups (4 heads each).
Per core: QKV projections (bf16, head-slice, inputs pre-transposed on host),
flash-style causal attention with no-max softmax (scores bounded ~+-3.4),
complexity bias folded into V rows as exp(-cpen*c_k), partial O-projection,
bf16 ReduceScatter over the 4-core batch group, residual + LayerNorm on the
local quarter of rows.
"""

import numpy as np
import ml_dtypes
from contextlib import ExitStack

import concourse.bass as bass
import concourse.bacc as bacc
import concourse.tile as tile
from concourse import mybir
from concourse.bass_utils import run_bass_kernel_spmd


class _Exec:
    """Cached jit executor mirroring bass2jax.run_bass_via_pjrt (axon path),
    so repeat kernel() calls skip retracing and host->device re-staging of
    unchanged inputs can be controlled by the caller."""

    def __init__(self, nc, n_cores=8):
        import jax
        from jax.sharding import Mesh, PartitionSpec
        from jax.experimental.shard_map import shard_map
        from concourse import bass2jax
        from concourse import mybir as mb

        bass2jax.install_neuronx_cc_hook()
        assert nc.dbg_addr is None
        partition_name = (nc.partition_id_tensor.name
                          if nc.partition_id_tensor else None)
        in_names, out_names, out_avals = [], [], []
        for alloc in nc.m.functions[0].allocations:
            if not isinstance(alloc, mb.MemoryLocationSet):
                continue
            name = alloc.memorylocations[0].name
            if alloc.kind == "ExternalInput":
                if name != partition_name:
                    in_names.append(name)
            elif alloc.kind == "ExternalOutput":
                shape = tuple(alloc.tensor_shape)
                dtype = mb.dt.np(alloc.dtype)
                out_names.append(name)
                out_avals.append(jax.core.ShapedArray(shape, dtype))
        self.nc = nc
        self.in_names = in_names
        self.out_names = out_names
        self.out_avals = out_avals
        self.n_cores = n_cores
        n_params = len(in_names)
        n_outs = len(out_names)
        donate = tuple(range(n_params, n_params + n_outs))
        all_names = in_names + out_names
        if partition_name is not None:
            all_names = all_names + [partition_name]

        def _body(*args):
            operands = list(args)
            if partition_name is not None:
                operands.append(bass2jax.partition_id_tensor())
            return tuple(bass2jax._bass_exec_p.bind(
                *operands,
                out_avals=tuple(out_avals),
                in_names=tuple(all_names),
                out_names=tuple(out_names),
                lowering_input_output_aliases=(),
                sim_require_finite=True,
                sim_require_nnan=True,
                nc=nc,
            ))

        devices = jax.devices()[:n_cores]
        self.mesh = Mesh(np.asarray(devices), ("core",))
        in_specs = (PartitionSpec("core"),) * (n_params + n_outs)
        out_specs = (PartitionSpec("core"),) * n_outs
        self.sharded = jax.jit(
            shard_map(_body, mesh=self.mesh, in_specs=in_specs,
                      out_specs=out_specs, check_rep=False),
            donate_argnums=donate, keep_unused=True)
        self._jax = jax

    def stage(self, in_maps):
        """Concatenate per-core inputs and move to devices; returns arg list."""
        import jax
        from jax.sharding import NamedSharding, PartitionSpec
        sh = NamedSharding(self.mesh, PartitionSpec("core"))
        args = []
        for name in self.in_names:
            cat = np.concatenate([np.asarray(m[name]) for m in in_maps], axis=0)
            args.append(jax.device_put(cat, sh))
        return args

    def zeros(self):
        import jax
        from jax.sharding import NamedSharding, PartitionSpec
        sh = NamedSharding(self.mesh, PartitionSpec("core"))
        return [jax.device_put(
                    np.zeros((self.n_cores * a.shape[0], *a.shape[1:]), a.dtype), sh)
                for a in self.out_avals]

    def run(self, staged_args, zeros=None):
        if zeros is None:
            zeros = self.zeros()
        outs = self.sharded(*staged_args, *zeros)
        self._jax.block_until_ready(outs)
        return outs

    def make_chain(self, n):
        """Jitted fn running the program n times back-to-back on device;
        outputs of call i feed the (ignored, fully-overwritten) output
        buffers of call i+1, forcing serial execution."""
        import jax
        from jax.sharding import PartitionSpec
        from jax.experimental.shard_map import shard_map
        from concourse import bass2jax

        partition_name = (self.nc.partition_id_tensor.name
                          if self.nc.partition_id_tensor else None)
        all_names = self.in_names + self.out_names
        if partition_name is not None:
            all_names = all_names + [partition_name]
        out_avals = self.out_avals
        nc = self.nc

        def _body(*args):
            n_params = len(self.in_names)
            ins = list(args[:n_params])
            z = list(args[n_params:])
            for _ in range(n):
                operands = ins + z
                if partition_name is not None:
                    operands.append(bass2jax.partition_id_tensor())
                outs = bass2jax._bass_exec_p.bind(
                    *operands,
                    out_avals=tuple(out_avals),
                    in_names=tuple(all_names),
                    out_names=tuple(self.out_names),
                    lowering_input_output_aliases=(),
                    sim_require_finite=True,
                    sim_require_nnan=True,
                    nc=nc,
                )
                z = list(outs)
            return tuple(z)

        n_io = len(self.in_names) + len(self.out_names)
        in_specs = (PartitionSpec("core"),) * n_io
        out_specs = (PartitionSpec("core"),) * len(self.out_names)
        return jax.jit(
            shard_map(_body, mesh=self.mesh, in_specs=in_specs,
                      out_specs=out_specs, check_rep=False),
            keep_unused=True)

    def results(self, outs):
        per_core = []
        for c in range(self.n_cores):
            d = {}
            for i, name in enumerate(self.out_names):
                a = self.out_avals[i]
                d[name] = np.asarray(outs[i]).reshape(
                    self.n_cores, *a.shape)[c]
            per_core.append(d)
        return per_core

# ---- problem constants (hardcoded per harness contract)
B, S, D, H = 2, 2048, 1024, 16
DK = D // H                      # 64
EPS = 1e-5
P = 128
HL = 4                           # heads per core
HD = HL * DK                     # 256 local head dims
SQ = S // 4                      # 512 output rows per core
NDT = D // P                     # 8 d-tiles
NQT = 4                          # q tiles of 512
QW = 512
NST = S // P                     # 16 s/k tiles
GROUPS = [[0, 1, 2, 3], [4, 5, 6, 7]]

f32 = mybir.dt.float32
bf16 = mybir.dt.bfloat16
f32r = mybir.dt.float32r
BF = ml_dtypes.bfloat16

Alu = mybir.AluOpType
Act = mybir.ActivationFunctionType

_BUILD_CACHE = {}

_ET_BUFS = 8  # exp-tile ring depth (absorbs ACT-vs-PE skew)


def _build(n_rep=1):
    key = ("nc", n_rep, _ET_BUFS)
    if key in _BUILD_CACHE:
        return _BUILD_CACHE[key]

    nc = bacc.Bacc("TRN2", target_bir_lowering=False, debug=False,
                   enable_asserts=False, num_devices=8)

    # ---- DRAM I/O (x inputs pre-transposed on host: [D, S])
    xqT_t = nc.dram_tensor("xqT", [D, S], bf16, kind="ExternalInput")
    xkT_t = nc.dram_tensor("xkT", [D, S], bf16, kind="ExternalInput")
    xvT_t = nc.dram_tensor("xvT", [D, S], bf16, kind="ExternalInput")
    wqT = nc.dram_tensor("wqT", [D, HD], bf16, kind="ExternalInput").ap()
    wkT = nc.dram_tensor("wkT", [D, HD], bf16, kind="ExternalInput").ap()
    wvT = nc.dram_tensor("wvT", [D, HD], bf16, kind="ExternalInput").ap()
    woT = nc.dram_tensor("woT", [HD, D], bf16, kind="ExternalInput").ap()
    bqk = nc.dram_tensor("bqk", [P, 4], f32, kind="ExternalInput").ap()
    bvm = nc.dram_tensor("bvm", [1, HD], bf16, kind="ExternalInput").ap()
    enb = nc.dram_tensor("enb", [P, NST], f32, kind="ExternalInput").ap()
    masks2 = nc.dram_tensor("masks2", [4, P, 2 * QW], bf16, kind="ExternalInput").ap()
    selp = nc.dram_tensor("selp", [HL, 2 * P], f32r, kind="ExternalInput").ap()
    xres = nc.dram_tensor("xres", [SQ, D], f32, kind="ExternalInput").ap()
    lngb = nc.dram_tensor("lngb", [P, D], f32, kind="ExternalInput").ap()
    lnbb = nc.dram_tensor("lnbb", [P, D], f32, kind="ExternalInput").ap()
    y = nc.dram_tensor("y", [SQ, D], f32, kind="ExternalOutput").ap()

    rs_in = nc.dram_tensor("rs_in", [S, D], bf16).ap()
    rs_out = nc.dram_tensor("rs_out", [SQ, D], bf16).ap()

    with tile.TileContext(nc) as tc, ExitStack() as ctx:
        consts = ctx.enter_context(tc.tile_pool(name="consts", bufs=1))
        xt_pool = ctx.enter_context(tc.tile_pool(name="xt", bufs=1))
        et_pool = ctx.enter_context(tc.tile_pool(name="et", bufs=_ET_BUFS))
        stg_pool = ctx.enter_context(tc.tile_pool(name="stg", bufs=2))
        out_pool = ctx.enter_context(tc.tile_pool(name="outp", bufs=2))
        ln_pool = ctx.enter_context(tc.tile_pool(name="ln", bufs=2))
        pp_ps = ctx.enter_context(tc.tile_pool(name="pp_ps", bufs=2, space="PSUM"))
        s_ps = ctx.enter_context(tc.tile_pool(name="s_ps", bufs=2, space="PSUM"))
        c_ps = ctx.enter_context(tc.tile_pool(name="c_ps", bufs=1, space="PSUM"))

        # ---- true constants (outside the timing loop)
        ones_sb = consts.tile([1, P], bf16, name="ones_sb")
        nc.vector.memset(ones_sb, 1.0)
        eps_sb = consts.tile([P, 1], f32, name="eps_sb")
        nc.vector.memset(eps_sb, EPS)
        mask_sb = []
        for j in range(4):
            t = consts.tile([P, 2 * QW], bf16, name=f"mask{j}")
            nc.sync.dma_start(out=t, in_=masks2[j])
            mask_sb.append(t)
        selp_sb = consts.tile([HL, 2 * P], f32r, name="selp_sb")
        nc.sync.dma_start(out=selp_sb, in_=selp)

        # persistent activation tiles
        QT_sb = [consts.tile([P, S], bf16, name=f"QT{g}") for g in range(2)]
        KT_sb = [consts.tile([P, S], bf16, name=f"KT{g}") for g in range(2)]
        # V tiles: [V(64), enb] per head
        V_sb = [consts.tile([P, HL, DK + 1], bf16, name=f"V{st}") for st in range(NST)]
        cpair = [consts.tile([P, S], f32, name=f"cp{pr}") for pr in range(2)]
        ctxN = [consts.tile([P, S], bf16, name=f"cn{pr}") for pr in range(2)]
        rsums = consts.tile([HL, S], f32, name="rsums")
        rsr = consts.tile([HL, S], f32r, name="rsr")
        # weight tiles (persistent slots; reloaded per iteration in-loop)
        wq_sb = [consts.tile([P, HD], bf16, name=f"wq{d}") for d in range(NDT)]
        wk_sb = [consts.tile([P, HD], bf16, name=f"wk{d}") for d in range(NDT)]
        wv_sb = [consts.tile([P, HD], bf16, name=f"wv{d}") for d in range(NDT)]
        wo_sb = [consts.tile([P, D], bf16, name=f"wo{j}") for j in range(2)]
        bqk_sb = consts.tile([P, 4], f32, name="bqk_sb")
        bvm_sb = consts.tile([1, HD], bf16, name="bvm_sb")
        enb_sb = consts.tile([P, NST], f32, name="enb_sb")

        def load_x(dst, src_t, nchunk, eng):
            """DMA [D, S] dram -> [128, NDT*S] sbuf in nchunk pieces."""
            nd = NDT // nchunk
            for c in range(nchunk):
                d0 = c * nd
                dst3 = dst[:, d0 * S:(d0 + nd) * S].rearrange(
                    "p (d s) -> p d s", d=nd)
                src3 = bass.AP(tensor=src_t, offset=d0 * P * S,
                               ap=[[S, P], [P * S, nd], [1, S]])
                eng.dma_start(out=dst3, in_=src3)

        def emit_body():
            # ---- input/weight loads.
            # scalar (ACT) HWDGE queue: wq, bqk, wk, xkT, wv, bvm, enb
            #   (everything gating QK proj + V proj; done before exp stream)
            # sync (SP) HWDGE queue: xqT, xvT, wo (then rsum/rs_in writes)
            for d in range(NDT):
                nc.scalar.dma_start(out=wq_sb[d], in_=wqT[d * P:(d + 1) * P, :])
            nc.scalar.dma_start(out=bqk_sb, in_=bqk)
            for d in range(NDT):
                nc.scalar.dma_start(out=wk_sb[d], in_=wkT[d * P:(d + 1) * P, :])

            xq_sb = xt_pool.tile([P, NDT * S], bf16, name="xq_sb", tag="xq")
            load_x(xq_sb, xqT_t, 4, nc.sync)
            xk_sb = xt_pool.tile([P, NDT * S], bf16, name="xk_sb", tag="xk")
            load_x(xk_sb, xkT_t, 2, nc.scalar)

            for d in range(NDT):
                nc.scalar.dma_start(out=wv_sb[d], in_=wvT[d * P:(d + 1) * P, :])
            nc.scalar.dma_start(out=bvm_sb, in_=bvm)
            nc.scalar.dma_start(out=enb_sb, in_=enb)

            xv_sb = xt_pool.tile([P, NDT * S], bf16, name="xv_sb", tag="xq")
            load_x(xv_sb, xvT_t, 2, nc.sync)
            for j in range(2):
                nc.sync.dma_start(out=wo_sb[j], in_=woT[j * P:(j + 1) * P, :])

            # ---- Q/K projections -> QT/KT [2][128(2 heads x 64), S]
            def qk_proj_g(x_sb, w_sb, bbase, out_sb, g):
                for q in range(NQT):
                    ps = pp_ps.tile([P, QW], f32, name="pp", tag="pp")
                    for d in range(NDT):
                        nc.tensor.matmul(
                            ps, lhsT=w_sb[d][:, g * P:(g + 1) * P],
                            rhs=x_sb[:, d * S + q * QW:d * S + (q + 1) * QW],
                            start=(d == 0), stop=(d == NDT - 1))
                    nc.vector.tensor_scalar(
                        out=out_sb[g][:, q * QW:(q + 1) * QW], in0=ps,
                        scalar1=bqk_sb[:, bbase + g:bbase + g + 1],
                        scalar2=None, op0=Alu.add)

            # ---- V projection (st tile group)
            def v_proj(sts):
                for st in sts:
                    ps = pp_ps.tile([P, QW], f32, name="ppv", tag="pp")
                    psv = ps[:, 0:HD]
                    nc.tensor.matmul(psv, lhsT=ones_sb[0:1, 0:P], rhs=bvm_sb,
                                     start=True, stop=False)
                    for d in range(NDT):
                        nc.tensor.matmul(
                            psv, lhsT=xv_sb[:, d * S + st * P:d * S + (st + 1) * P],
                            rhs=wv_sb[d],
                            start=False, stop=(d == NDT - 1))
                    psr = psv.rearrange("p (h c) -> p h c", h=HL)
                    nc.vector.tensor_scalar(
                        out=V_sb[st][:, :, 0:DK], in0=psr,
                        scalar1=enb_sb[:, st:st + 1], scalar2=None, op0=Alu.mult)
                    nc.vector.tensor_copy(
                        out=V_sb[st][:, :, DK:DK + 1],
                        in_=enb_sb[:, st:st + 1].to_broadcast([P, HL, 1]))

            # ---- attention for one (pair, q-tile); pre_ctx() emits work
            # (e.g. the V projection group) between the scores/exp stream
            # and the ctx matmuls that consume V
            def att_q(pr, q, pre_ctx=None):
                    g = pr
                    nkt = 4 * (q + 1)
                    qs = slice(q * QW, (q + 1) * QW)
                    ctxA_t = c_ps.tile([P, QW], f32, name="ctxA", tag="cpA")
                    ctxB_t = c_ps.tile([P, QW], f32, name="ctxB", tag="cpB")
                    ctxA = ctxA_t[0:DK + 1, :]
                    ets = []
                    for kt in range(nkt):
                        j = kt - 4 * q  # >=0 on diagonal group
                        rs0 = 128 * j if j >= 0 else 0
                        # scoresT for both heads of the pair in one pass:
                        # 4 concurrent [K=64, M=64, N<=512] quadrant matmuls.
                        ps = s_ps.tile([P, 2 * QW], f32, name="sc", tag="sp")
                        for hh in range(2):
                            ho = hh * 64
                            for jj in range(2):
                                nc.tensor.matmul(
                                    ps[jj * 64:(jj + 1) * 64,
                                       hh * QW + rs0:(hh + 1) * QW],
                                    lhsT=KT_sb[g][ho:ho + 64,
                                                  kt * P + jj * 64:
                                                  kt * P + (jj + 1) * 64],
                                    rhs=QT_sb[g][ho:ho + 64,
                                                 q * QW + rs0:(q + 1) * QW],
                                    start=True, stop=True,
                                    tile_position=(ho, jj * 64))
                        et = et_pool.tile([P, 2, QW], bf16, name="et", tag="et")
                        ps3 = ps.rearrange("p (h f) -> p h f", h=2)
                        nc.scalar.activation(out=et[:, :, rs0:QW],
                                             in_=ps3[:, :, rs0:QW],
                                             func=Act.Exp, scale=0.125)
                        if j >= 0:
                            nc.vector.tensor_mul(
                                et[:, :, rs0:QW],
                                et[:, :, rs0:QW],
                                mask_sb[j].rearrange(
                                    "p (h f) -> p h f", h=2)[:, :, rs0:QW])
                        ets.append((et, rs0))
                    if pre_ctx is not None:
                        pre_ctx()
                    for kt, (et, rs0) in enumerate(ets):
                        nc.tensor.matmul(
                            ctxA[:, rs0:QW], lhsT=V_sb[kt][:, 2 * pr, :],
                            rhs=et[:, 0, rs0:QW],
                            start=(kt == 0), stop=(kt == nkt - 1))
                        # odd head: V rows -> partitions 64:128 (col groups
                        # 2-3); its enb sum row -> partition 32 (col group 1,
                        # runs concurrent with the V matmul)
                        nc.tensor.matmul(
                            ctxB_t[64:P, rs0:QW],
                            lhsT=V_sb[kt][:, 2 * pr + 1, 0:DK],
                            rhs=et[:, 1, rs0:QW],
                            start=(kt == 0), stop=(kt == nkt - 1),
                            tile_position=(0, 64))
                        nc.tensor.matmul(
                            ctxB_t[32:33, rs0:QW],
                            lhsT=V_sb[kt][:, 2 * pr + 1, DK:DK + 1],
                            rhs=et[:, 1, rs0:QW],
                            start=(kt == 0), stop=(kt == nkt - 1),
                            tile_position=(0, 32))
                    # ctxA rows: V@0:64, sum@64 ; ctxB rows: sum@32, V@64:128
                    nc.vector.tensor_copy(out=cpair[pr][0:DK, qs],
                                          in_=ctxA[0:DK, :])
                    nc.vector.tensor_copy(out=cpair[pr][64:P, qs],
                                          in_=ctxB_t[64:P, :])
                    srow = stg_pool.tile([P, QW], f32, name="srow", tag="sr")
                    nc.vector.tensor_copy(out=srow[64:65, :],
                                          in_=ctxA_t[DK:DK + 1, :])
                    nc.vector.tensor_copy(out=srow[32:33, :],
                                          in_=ctxB_t[32:33, :])
                    nc.sync.dma_start(out=rsums[2 * pr:2 * pr + 1, qs],
                                      in_=srow[64:65, :])
                    nc.sync.dma_start(out=rsums[2 * pr + 1:2 * pr + 2, qs],
                                      in_=srow[32:33, :])

            # ---- orchestration: all of Q first (so the xv load, which
            # shares the xq buffer, can start), then K g0 unblocks pair-0
            # attention; V-projection groups and the g1 K-projection hide
            # under pair-0's exp stream.
            qk_proj_g(xq_sb, wq_sb, 0, QT_sb, 0)
            qk_proj_g(xq_sb, wq_sb, 0, QT_sb, 1)
            qk_proj_g(xk_sb, wk_sb, 2, KT_sb, 0)
            for q in range(NQT):
                att_q(0, q,
                      pre_ctx=(lambda q=q: v_proj(range(4 * q, 4 * q + 4))))
            qk_proj_g(xk_sb, wk_sb, 2, KT_sb, 1)
            for q in range(NQT):
                att_q(1, q)

            # ---- normalize ctx -> ctxN (bf16)
            nc.vector.reciprocal(out=rsums, in_=rsums)
            nc.vector.tensor_copy(out=rsr, in_=rsums)
            for pr in range(2):
                for q in range(NQT):
                    qs = slice(q * QW, (q + 1) * QW)
                    psb = pp_ps.tile([P, QW], f32, name="bcps", tag="pp")
                    nc.tensor.matmul(
                        psb,
                        lhsT=selp_sb[:, pr * P:(pr + 1) * P],
                        rhs=rsr[:, qs],
                        start=True, stop=True)
                    nc.vector.tensor_mul(ctxN[pr][:, qs], cpair[pr][:, qs], psb)

            # ---- O projection -> rs_in (bf16; one DMA per q-block,
            # alternating between the two HWDGE queues)
            for qb in range(NST):
                o_sb = out_pool.tile([P, 2 * QW], bf16, name="o_sb", tag="ob")
                for dh in range(2):
                    ps = pp_ps.tile([P, QW], f32, name="ops", tag="pp")
                    for pr in range(2):
                        nc.tensor.matmul(
                            ps, lhsT=ctxN[pr][:, qb * P:(qb + 1) * P],
                            rhs=wo_sb[pr][:, dh * QW:(dh + 1) * QW],
                            start=(pr == 0), stop=(pr == 1))
                    if dh == 0:
                        nc.vector.tensor_copy(out=o_sb[:, 0:QW], in_=ps)
                    else:
                        nc.scalar.copy(out=o_sb[:, QW:2 * QW], in_=ps)
                eng = nc.sync if qb % 2 == 0 else nc.scalar
                eng.dma_start(out=rs_in[qb * P:(qb + 1) * P, :], in_=o_sb)

        def emit_finish():
            # ---- ReduceScatter over the batch group (bf16)
            nc.gpsimd.collective_compute(
                "ReduceScatter", Alu.add, replica_groups=GROUPS,
                ins=[rs_in.opt()], outs=[rs_out.opt()])

            lng_sb = consts.tile([P, D], f32, name="lng_sb")
            nc.scalar.dma_start(out=lng_sb, in_=lngb)
            lnb_sb = consts.tile([P, D], f32, name="lnb_sb")
            nc.scalar.dma_start(out=lnb_sb, in_=lnbb)

            # ---- residual + LayerNorm on local rows
            for t in range(SQ // P):
                rsl = slice(t * P, (t + 1) * P)
                rs_sb = ln_pool.tile([P, D], bf16, name="rs_sb", tag="lrs")
                nc.sync.dma_start(out=rs_sb, in_=rs_out[rsl, :])
                xr_sb = ln_pool.tile([P, D], f32, name="xr_sb", tag="lr")
                nc.sync.dma_start(out=xr_sb, in_=xres[rsl, :])
                x_sb = ln_pool.tile([P, D], f32, name="x_sb", tag="lx")
                nc.vector.tensor_copy(out=x_sb, in_=rs_sb)
                nc.vector.tensor_add(x_sb, x_sb, xr_sb)
                stats = ln_pool.tile([P, 2, 6], f32, name="stats", tag="lst")
                for sg in range(2):
                    nc.vector.bn_stats(out=stats[:, sg, :],
                                       in_=x_sb[:, sg * QW:(sg + 1) * QW])
                mv = ln_pool.tile([P, 2], f32, name="mv", tag="lmv")
                nc.vector.bn_aggr(out=mv, in_=stats)
                nc.scalar.activation(out=mv[:, 1:2], in_=mv[:, 1:2],
                                     func=Act.Sqrt, bias=eps_sb, scale=1.0)
                nc.vector.reciprocal(out=mv[:, 1:2], in_=mv[:, 1:2])
                nc.vector.tensor_scalar(
                    out=x_sb, in0=x_sb, scalar1=mv[:, 0:1], scalar2=mv[:, 1:2],
                    op0=Alu.subtract, op1=Alu.mult)
                nc.vector.tensor_mul(x_sb, x_sb, lng_sb)
                nc.vector.tensor_add(x_sb, x_sb, lnb_sb)
                nc.sync.dma_start(out=y[rsl, :], in_=x_sb)

        if n_rep == 1:
            emit_body()
        else:
            with tc.For_i(0, n_rep, 1):
                emit_body()
        emit_finish()

    nc.compile()
    _BUILD_CACHE[key] = nc
    return nc


def _make_masks():
    # mask[j][p, half*512 + f] = 1.0 if p + j*128 <= f else 0 (same both halves;
    # halves hold the two heads of a pair for the same k-tile)
    m = np.zeros((4, P, 2 * QW), dtype=np.float32)
    p = np.arange(P)[:, None]
    f = np.arange(QW)[None, :]
    for j in range(4):
        o = j * P
        keep = (p + o <= f)
        m[j][:, 0:QW] = keep
        m[j][:, QW:2 * QW] = keep
    return m.astype(BF)


def _make_selp():
    sp = np.zeros((HL, 2 * P), dtype=np.float32)
    mm = np.arange(P)
    for pr in range(2):
        for k in range(HL):
            sp[k, pr * P:(pr + 1) * P] = (k == 2 * pr + mm // 64)
    return sp


def _prep_in_maps(query, key, value, complexity, wq, bq, wk, bk, wv, bv,
                  wo, bo, ln_g, ln_b, cpen):
    masks2 = _make_masks()
    selp = _make_selp()
    lngb = np.ascontiguousarray(
        np.broadcast_to(np.asarray(ln_g, np.float32)[None, :], (P, D)))
    lnbb = np.ascontiguousarray(
        np.broadcast_to(np.asarray(ln_b, np.float32)[None, :], (P, D)))
    per_batch = []
    for b in range(B):
        xqT = np.ascontiguousarray(query[b].astype(BF).T)
        xkT = np.ascontiguousarray(key[b].astype(BF).T)
        xvT = np.ascontiguousarray(value[b].astype(BF).T)
        e = np.exp(-float(cpen) * complexity[b].astype(np.float64)).astype(np.float32)
        enb_l = np.ascontiguousarray(e.reshape(NST, P).T)
        per_batch.append((xqT, xkT, xvT, enb_l))
    in_maps = []
    for c in range(8):
        b, r = c // 4, c % 4
        hs = HD * r
        xqT, xkT, xvT, enb_l = per_batch[b]
        bqc = bq[hs:hs + HD].astype(np.float32).reshape(2, P).T  # [P, 2]
        bkc = bk[hs:hs + HD].astype(np.float32).reshape(2, P).T
        bqk_l = np.ascontiguousarray(
            np.concatenate([bqc, bkc], axis=1))  # [P,4]: q_g0,q_g1,k_g0,k_g1
        in_maps.append({
            "xqT": xqT, "xkT": xkT, "xvT": xvT,
            "wqT": np.ascontiguousarray(wq[hs:hs + HD, :].T).astype(BF),
            "wkT": np.ascontiguousarray(wk[hs:hs + HD, :].T).astype(BF),
            "wvT": np.ascontiguousarray(wv[hs:hs + HD, :].T).astype(BF),
            "woT": np.ascontiguousarray(wo[:, hs:hs + HD].T).astype(BF),
            "bqk": bqk_l,
            "bvm": bv[hs:hs + HD].astype(BF)[None, :],
            "enb": enb_l,
            "masks2": masks2,
            "selp": selp,
            "xres": (query[b][SQ * r:SQ * (r + 1)].astype(np.float32)
                     + bo.astype(np.float32)[None, :]),
            "lngb": lngb, "lnbb": lnbb,
        })
    return in_maps


def _numpy_fallback(query, key, value, complexity, mask, wq, bq, wk, bk,
                    wv, bv, wo, bo, ln_g, ln_b, cpen):
    import math
    out = np.zeros((B, S, D), np.float32)
    for b in range(B):
        Q = query[b] @ wq.T + bq
        K = key[b] @ wk.T + bk
        V = value[b] @ wv.T + bv
        Qh = Q.reshape(S, H, DK).transpose(1, 0, 2)
        Kh = K.reshape(S, H, DK).transpose(1, 0, 2)
        Vh = V.reshape(S, H, DK).transpose(1, 0, 2)
        ctx = np.zeros((H, S, DK), np.float32)
        m = mask[b, 0]
        for h in range(H):
            sc = Qh[h] @ Kh[h].T / math.sqrt(DK) - cpen * complexity[b][None, :]
            sc = np.where(m, sc, -1e9)
            sc = sc - sc.max(-1, keepdims=True)
            e = np.exp(sc)
            a = e / e.sum(-1, keepdims=True)
            ctx[h] = a @ Vh[h]
        x = ctx.transpose(1, 0, 2).reshape(S, D) @ wo.T + bo + query[b]
        mu = x.mean(-1, keepdims=True)
        var = ((x - mu) ** 2).mean(-1, keepdims=True)
        out[b] = (x - mu) / np.sqrt(var + EPS) * ln_g + ln_b
    return out


_TRIL = None


def kernel(query, key, value, complexity, mask, wq, bq, wk, bk, wv, bv,
           wo, bo, ln_g, ln_b, cpen, **_unused):
    query = np.asarray(query, dtype=np.float32)
    key = np.asarray(key, dtype=np.float32)
    value = np.asarray(value, dtype=np.float32)
    complexity = np.asarray(complexity, dtype=np.float32)
    mask = np.asarray(mask)
    args = dict(query=query, key=key, value=value, complexity=complexity,
                wq=np.asarray(wq), bq=np.asarray(bq), wk=np.asarray(wk),
                bk=np.asarray(bk), wv=np.asarray(wv), bv=np.asarray(bv),
                wo=np.asarray(wo), bo=np.asarray(bo),
                ln_g=np.asarray(ln_g), ln_b=np.asarray(ln_b),
                cpen=float(np.asarray(cpen)))
    global _TRIL
    if _TRIL is None:
        _TRIL = np.tril(np.ones((S, S), bool))
    if not all(np.array_equal(mask[b, 0], _TRIL) for b in range(B)):
        # non-causal mask: fall back to a generic host implementation
        return _numpy_fallback(mask=mask, **args)

    ex = _get_exec()
    in_maps = _prep_in_maps(**args)
    outs = ex.run(ex.stage(in_maps))
    res = ex.results(outs)
    out = np.empty((B, S, D), np.float32)
    for c in range(8):
        b, r = c // 4, c % 4
        out[b, SQ * r:SQ * (r + 1)] = res[c]["y"]
    return out


def _get_exec():
    if "ex" not in _BUILD_CACHE:
        _BUILD_CACHE["ex"] = _Exec(_build())
    return _BUILD_CACHE["ex"]


def _input_args(inputs):
    return dict(query=np.asarray(inputs["query"], np.float32),
                key=np.asarray(inputs["key"], np.float32),
                value=np.asarray(inputs["value"], np.float32),
                complexity=np.asarray(inputs["complexity"], np.float32),
                wq=np.asarray(inputs["wq"]), bq=np.asarray(inputs["bq"]),
                wk=np.asarray(inputs["wk"]), bk=np.asarray(inputs["bk"]),
                wv=np.asarray(inputs["wv"]), bv=np.asarray(inputs["bv"]),
                wo=np.asarray(inputs["wo"]), bo=np.asarray(inputs["bo"]),
                ln_g=np.asarray(inputs["ln_g"]), ln_b=np.asarray(inputs["ln_b"]),
                cpen=float(np.asarray(inputs["cpen"])))


# revision 105
# speedup vs baseline: 1.2940x; 1.0622x over previous
"""Trainium2 Bass kernel: ComplexityAwareAttention (B=2, S=2048, D=1024, H=16).

Sharding: 8 cores = 2 batches x 4 head-groups (4 heads each).
Per core: QKV projections (bf16, head-slice, inputs pre-transposed on host),
flash-style causal attention with no-max softmax (scores bounded ~+-3.4),
complexity bias folded into V rows as exp(-cpen*c_k), partial O-projection,
bf16 ReduceScatter over the 4-core batch group, residual + LayerNorm on the
local quarter of rows.
"""

import numpy as np
import ml_dtypes
from contextlib import ExitStack

import concourse.bass as bass
import concourse.bacc as bacc
import concourse.tile as tile
from concourse import mybir
from concourse.bass_utils import run_bass_kernel_spmd


class _Exec:
    """Cached jit executor mirroring bass2jax.run_bass_via_pjrt (axon path),
    so repeat kernel() calls skip retracing and host->device re-staging of
    unchanged inputs can be controlled by the caller."""

    def __init__(self, nc, n_cores=8):
        import jax
        from jax.sharding import Mesh, PartitionSpec
        from jax.experimental.shard_map import shard_map
        from concourse import bass2jax
        from concourse import mybir as mb

        bass2jax.install_neuronx_cc_hook()
        assert nc.dbg_addr is None
        partition_name = (nc.partition_id_tensor.name
                          if nc.partition_id_tensor else None)
        in_names, out_names, out_avals = [], [], []
        for alloc in nc.m.functions[0].allocations:
            if not isinstance(alloc, mb.MemoryLocationSet):
                continue
            name = alloc.memorylocations[0].name
            if alloc.kind == "ExternalInput":
                if name != partition_name:
                    in_names.append(name)
            elif alloc.kind == "ExternalOutput":
                shape = tuple(alloc.tensor_shape)
                dtype = mb.dt.np(alloc.dtype)
                out_names.append(name)
                out_avals.append(jax.core.ShapedArray(shape, dtype))
        self.nc = nc
        self.in_names = in_names
        self.out_names = out_names
        self.out_avals = out_avals
        self.n_cores = n_cores
        n_params = len(in_names)
        n_outs = len(out_names)
        donate = tuple(range(n_params, n_params + n_outs))
        all_names = in_names + out_names
        if partition_name is not None:
            all_names = all_names + [partition_name]

        def _body(*args):
            operands = list(args)
            if partition_name is not None:
                operands.append(bass2jax.partition_id_tensor())
            return tuple(bass2jax._bass_exec_p.bind(
                *operands,
                out_avals=tuple(out_avals),
                in_names=tuple(all_names),
                out_names=tuple(out_names),
                lowering_input_output_aliases=(),
                sim_require_finite=True,
                sim_require_nnan=True,
                nc=nc,
            ))

        devices = jax.devices()[:n_cores]
        self.mesh = Mesh(np.asarray(devices), ("core",))
        in_specs = (PartitionSpec("core"),) * (n_params + n_outs)
        out_specs = (PartitionSpec("core"),) * n_outs
        self.sharded = jax.jit(
            shard_map(_body, mesh=self.mesh, in_specs=in_specs,
                      out_specs=out_specs, check_rep=False),
            donate_argnums=donate, keep_unused=True)
        self._jax = jax

    def stage(self, in_maps):
        """Concatenate per-core inputs and move to devices; returns arg list."""
        import jax
        from jax.sharding import NamedSharding, PartitionSpec
        sh = NamedSharding(self.mesh, PartitionSpec("core"))
        args = []
        for name in self.in_names:
            cat = np.concatenate([np.asarray(m[name]) for m in in_maps], axis=0)
            args.append(jax.device_put(cat, sh))
        return args

    def zeros(self):
        import jax
        from jax.sharding import NamedSharding, PartitionSpec
        sh = NamedSharding(self.mesh, PartitionSpec("core"))
        return [jax.device_put(
                    np.zeros((self.n_cores * a.shape[0], *a.shape[1:]), a.dtype), sh)
                for a in self.out_avals]

    def run(self, staged_args, zeros=None):
        if zeros is None:
            zeros = self.zeros()
        outs = self.sharded(*staged_args, *zeros)
        self._jax.block_until_ready(outs)
        return outs

    def make_chain(self, n):
        """Jitted fn running the program n times back-to-back on device;
        outputs of call i feed the (ignored, fully-overwritten) output
        buffers of call i+1, forcing serial execution."""
        import jax
        from jax.sharding import PartitionSpec
        from jax.experimental.shard_map import shard_map
        from concourse import bass2jax

        partition_name = (self.nc.partition_id_tensor.name
                          if self.nc.partition_id_tensor else None)
        all_names = self.in_names + self.out_names
        if partition_name is not None:
            all_names = all_names + [partition_name]
        out_avals = self.out_avals
        nc = self.nc

        def _body(*args):
            n_params = len(self.in_names)
            ins = list(args[:n_params])
            z = list(args[n_params:])
            for _ in range(n):
                operands = ins + z
                if partition_name is not None:
                    operands.append(bass2jax.partition_id_tensor())
                outs = bass2jax._bass_exec_p.bind(
                    *operands,
                    out_avals=tuple(out_avals),
                    in_names=tuple(all_names),
                    out_names=tuple(self.out_names),
                    lowering_input_output_aliases=(),
                    sim_require_finite=True,
                    sim_require_nnan=True,
                    nc=nc,
                )
                z = list(outs)
            return tuple(z)

        n_io = len(self.in_names) + len(self.out_names)
        in_specs = (PartitionSpec("core"),) * n_io
        out_specs = (PartitionSpec("core"),) * len(self.out_names)
        return jax.jit(
            shard_map(_body, mesh=self.mesh, in_specs=in_specs,
                      out_specs=out_specs, check_rep=False),
            keep_unused=True)

    def results(self, outs):
        per_core = []
        for c in range(self.n_cores):
            d = {}
            for i, name in enumerate(self.out_names):
                a = self.out_avals[i]
                d[name] = np.asarray(outs[i]).reshape(
                    self.n_cores, *a.shape)[c]
            per_core.append(d)
        return per_core

# ---- problem constants (hardcoded per harness contract)
B, S, D, H = 2, 2048, 1024, 16
DK = D // H                      # 64
EPS = 1e-5
P = 128
HL = 4                           # heads per core
HD = HL * DK                     # 256 local head dims
SQ = S // 4                      # 512 output rows per core
NDT = D // P                     # 8 d-tiles
NQT = 4                          # q tiles of 512
QW = 512
NST = S // P                     # 16 s/k tiles
GROUPS = [[0, 1, 2, 3], [4, 5, 6, 7]]

f32 = mybir.dt.float32
bf16 = mybir.dt.bfloat16
f32r = mybir.dt.float32r
BF = ml_dtypes.bfloat16

Alu = mybir.AluOpType
Act = mybir.ActivationFunctionType

_BUILD_CACHE = {}

_ET_BUFS = 8  # exp-tile ring depth (absorbs ACT-vs-PE skew)


def _build(n_rep=1):
    key = ("nc", n_rep, _ET_BUFS)
    if key in _BUILD_CACHE:
        return _BUILD_CACHE[key]

    nc = bacc.Bacc("TRN2", target_bir_lowering=False, debug=False,
                   enable_asserts=False, num_devices=8)

    # ---- DRAM I/O (x inputs pre-transposed on host: [D, S])
    xqT_t = nc.dram_tensor("xqT", [D, S], bf16, kind="ExternalInput")
    xkT_t = nc.dram_tensor("xkT", [D, S], bf16, kind="ExternalInput")
    xvT_t = nc.dram_tensor("xvT", [D, S], bf16, kind="ExternalInput")
    wqT = nc.dram_tensor("wqT", [D, HD], bf16, kind="ExternalInput").ap()
    wkT = nc.dram_tensor("wkT", [D, HD], bf16, kind="ExternalInput").ap()
    wvT = nc.dram_tensor("wvT", [D, HD], bf16, kind="ExternalInput").ap()
    woT = nc.dram_tensor("woT", [HD, D], bf16, kind="ExternalInput").ap()
    bqk = nc.dram_tensor("bqk", [P, 4], f32, kind="ExternalInput").ap()
    bvm = nc.dram_tensor("bvm", [1, HD], bf16, kind="ExternalInput").ap()
    enb = nc.dram_tensor("enb", [P, NST], f32, kind="ExternalInput").ap()
    masks2 = nc.dram_tensor("masks2", [4, P, 2 * QW], bf16, kind="ExternalInput").ap()
    selp = nc.dram_tensor("selp", [HL, 2 * P], f32r, kind="ExternalInput").ap()
    xres = nc.dram_tensor("xres", [SQ, D], f32, kind="ExternalInput").ap()
    lngb = nc.dram_tensor("lngb", [P, D], f32, kind="ExternalInput").ap()
    lnbb = nc.dram_tensor("lnbb", [P, D], f32, kind="ExternalInput").ap()
    y = nc.dram_tensor("y", [SQ, D], f32, kind="ExternalOutput").ap()

    rs_in = nc.dram_tensor("rs_in", [S, D], bf16).ap()
    rs_out = nc.dram_tensor("rs_out", [SQ, D], bf16).ap()

    with tile.TileContext(nc) as tc, ExitStack() as ctx:
        consts = ctx.enter_context(tc.tile_pool(name="consts", bufs=1))
        xt_pool = ctx.enter_context(tc.tile_pool(name="xt", bufs=1))
        et_pool = ctx.enter_context(tc.tile_pool(name="et", bufs=_ET_BUFS))
        stg_pool = ctx.enter_context(tc.tile_pool(name="stg", bufs=2))
        out_pool = ctx.enter_context(tc.tile_pool(name="outp", bufs=2))
        ln_pool = ctx.enter_context(tc.tile_pool(name="ln", bufs=2))
        pp_ps = ctx.enter_context(tc.tile_pool(name="pp_ps", bufs=2, space="PSUM"))
        s_ps = ctx.enter_context(tc.tile_pool(name="s_ps", bufs=2, space="PSUM"))
        c_ps = ctx.enter_context(tc.tile_pool(name="c_ps", bufs=1, space="PSUM"))

        # ---- true constants (outside the timing loop)
        ones_sb = consts.tile([1, P], bf16, name="ones_sb")
        nc.vector.memset(ones_sb, 1.0)
        eps_sb = consts.tile([P, 1], f32, name="eps_sb")
        nc.vector.memset(eps_sb, EPS)
        mask_sb = []
        for j in range(4):
            t = consts.tile([P, 2 * QW], bf16, name=f"mask{j}")
            nc.sync.dma_start(out=t, in_=masks2[j])
            mask_sb.append(t)
        selp_sb = consts.tile([HL, 2 * P], f32r, name="selp_sb")
        nc.sync.dma_start(out=selp_sb, in_=selp)

        # persistent activation tiles
        QT_sb = [consts.tile([P, S], bf16, name=f"QT{g}") for g in range(2)]
        KT_sb = [consts.tile([P, S], bf16, name=f"KT{g}") for g in range(2)]
        # V tiles: [V(64), enb] per head
        V_sb = [consts.tile([P, HL, DK + 1], bf16, name=f"V{st}") for st in range(NST)]
        cpair = [consts.tile([P, S], f32, name=f"cp{pr}") for pr in range(2)]
        ctxN = [consts.tile([P, S], bf16, name=f"cn{pr}") for pr in range(2)]
        rsums = consts.tile([HL, S], f32, name="rsums")
        rsr = consts.tile([HL, S], f32r, name="rsr")
        # weight tiles (persistent slots; reloaded per iteration in-loop)
        wq_sb = [consts.tile([P, HD], bf16, name=f"wq{d}") for d in range(NDT)]
        wk_sb = [consts.tile([P, HD], bf16, name=f"wk{d}") for d in range(NDT)]
        wv_sb = [consts.tile([P, HD], bf16, name=f"wv{d}") for d in range(NDT)]
        wo_sb = [consts.tile([P, D], bf16, name=f"wo{j}") for j in range(2)]
        bqk_sb = consts.tile([P, 4], f32, name="bqk_sb")
        bvm_sb = consts.tile([1, HD], bf16, name="bvm_sb")
        enb_sb = consts.tile([P, NST], f32, name="enb_sb")

        def load_x(dst, src_t, nchunk, eng):
            """DMA [D, S] dram -> [128, NDT*S] sbuf in nchunk pieces."""
            nd = NDT // nchunk
            for c in range(nchunk):
                d0 = c * nd
                dst3 = dst[:, d0 * S:(d0 + nd) * S].rearrange(
                    "p (d s) -> p d s", d=nd)
                src3 = bass.AP(tensor=src_t, offset=d0 * P * S,
                               ap=[[S, P], [P * S, nd], [1, S]])
                eng.dma_start(out=dst3, in_=src3)

        def emit_body():
            # ---- input/weight loads.
            # scalar (ACT) HWDGE queue: wq, bqk, wk, xkT, wv, bvm, enb
            #   (everything gating QK proj + V proj; done before exp stream)
            # sync (SP) HWDGE queue: xqT, xvT, wo (then rsum/rs_in writes)
            for d in range(NDT):
                nc.scalar.dma_start(out=wq_sb[d], in_=wqT[d * P:(d + 1) * P, :])
            nc.scalar.dma_start(out=bqk_sb, in_=bqk)
            for d in range(NDT):
                nc.scalar.dma_start(out=wk_sb[d], in_=wkT[d * P:(d + 1) * P, :])

            xq_sb = xt_pool.tile([P, NDT * S], bf16, name="xq_sb", tag="xq")
            load_x(xq_sb, xqT_t, 4, nc.sync)
            xk_sb = xt_pool.tile([P, NDT * S], bf16, name="xk_sb", tag="xk")
            load_x(xk_sb, xkT_t, 2, nc.scalar)

            for d in range(NDT):
                nc.scalar.dma_start(out=wv_sb[d], in_=wvT[d * P:(d + 1) * P, :])
            nc.scalar.dma_start(out=bvm_sb, in_=bvm)
            nc.scalar.dma_start(out=enb_sb, in_=enb)

            xv_sb = xt_pool.tile([P, NDT * S], bf16, name="xv_sb", tag="xq")
            load_x(xv_sb, xvT_t, 2, nc.sync)
            for j in range(2):
                nc.sync.dma_start(out=wo_sb[j], in_=woT[j * P:(j + 1) * P, :])

            # ---- Q/K projections -> QT/KT [2][128(2 heads x 64), S]
            def qk_proj_g(x_sb, w_sb, bbase, out_sb, g):
                for q in range(NQT):
                    ps = pp_ps.tile([P, QW], f32, name="pp", tag="pp")
                    for d in range(NDT):
                        nc.tensor.matmul(
                            ps, lhsT=w_sb[d][:, g * P:(g + 1) * P],
                            rhs=x_sb[:, d * S + q * QW:d * S + (q + 1) * QW],
                            start=(d == 0), stop=(d == NDT - 1))
                    nc.vector.tensor_scalar(
                        out=out_sb[g][:, q * QW:(q + 1) * QW], in0=ps,
                        scalar1=bqk_sb[:, bbase + g:bbase + g + 1],
                        scalar2=None, op0=Alu.add)

            # ---- V projection (st tile group)
            def v_proj(sts):
                for st in sts:
                    ps = pp_ps.tile([P, QW], f32, name="ppv", tag="pp")
                    psv = ps[:, 0:HD]
                    nc.tensor.matmul(psv, lhsT=ones_sb[0:1, 0:P], rhs=bvm_sb,
                                     start=True, stop=False)
                    for d in range(NDT):
                        nc.tensor.matmul(
                            psv, lhsT=xv_sb[:, d * S + st * P:d * S + (st + 1) * P],
                            rhs=wv_sb[d],
                            start=False, stop=(d == NDT - 1))
                    psr = psv.rearrange("p (h c) -> p h c", h=HL)
                    nc.vector.tensor_scalar(
                        out=V_sb[st][:, :, 0:DK], in0=psr,
                        scalar1=enb_sb[:, st:st + 1], scalar2=None, op0=Alu.mult)
                    nc.vector.tensor_copy(
                        out=V_sb[st][:, :, DK:DK + 1],
                        in_=enb_sb[:, st:st + 1].to_broadcast([P, HL, 1]))

            # ---- attention for one (pair, q-tile); pre_ctx() emits work
            # (e.g. the V projection group) between the scores/exp stream
            # and the ctx matmuls that consume V
            def att_q(pr, q, pre_ctx=None):
                    g = pr
                    nkt = 4 * (q + 1)
                    qs = slice(q * QW, (q + 1) * QW)
                    ctxA_t = c_ps.tile([P, QW], f32, name="ctxA", tag="cpA")
                    ctxB_t = c_ps.tile([P, QW], f32, name="ctxB", tag="cpB")
                    ctxA = ctxA_t[0:DK + 1, :]
                    ets = []
                    for kt in range(nkt):
                        j = kt - 4 * q  # >=0 on diagonal group
                        rs0 = 128 * j if j >= 0 else 0
                        # scoresT for both heads of the pair: 2 row-tiled
                        # [K=64, M=128, N<=512] matmuls (PE row halves) —
                        # half the streaming cycles of the 4-quadrant form
                        ps = s_ps.tile([P, 2 * QW], f32, name="sc", tag="sp")
                        for hh in range(2):
                            ho = hh * 64
                            nc.tensor.matmul(
                                ps[:, hh * QW + rs0:(hh + 1) * QW],
                                lhsT=KT_sb[g][ho:ho + 64,
                                              kt * P:(kt + 1) * P],
                                rhs=QT_sb[g][ho:ho + 64,
                                             q * QW + rs0:(q + 1) * QW],
                                start=True, stop=True,
                                tile_position=(ho, 0))
                        et = et_pool.tile([P, 2, QW], bf16, name="et", tag="et")
                        ps3 = ps.rearrange("p (h f) -> p h f", h=2)
                        nc.scalar.activation(out=et[:, :, rs0:QW],
                                             in_=ps3[:, :, rs0:QW],
                                             func=Act.Exp, scale=0.125)
                        if j >= 0:
                            nc.vector.tensor_mul(
                                et[:, :, rs0:QW],
                                et[:, :, rs0:QW],
                                mask_sb[j].rearrange(
                                    "p (h f) -> p h f", h=2)[:, :, rs0:QW])
                        ets.append((et, rs0))
                    if pre_ctx is not None:
                        pre_ctx()
                    for kt, (et, rs0) in enumerate(ets):
                        nc.tensor.matmul(
                            ctxA[:, rs0:QW], lhsT=V_sb[kt][:, 2 * pr, :],
                            rhs=et[:, 0, rs0:QW],
                            start=(kt == 0), stop=(kt == nkt - 1))
                        # odd head: V rows -> partitions 64:128 (col groups
                        # 2-3); its enb sum row -> partition 32 (col group 1,
                        # runs concurrent with the V matmul)
                        nc.tensor.matmul(
                            ctxB_t[64:P, rs0:QW],
                            lhsT=V_sb[kt][:, 2 * pr + 1, 0:DK],
                            rhs=et[:, 1, rs0:QW],
                            start=(kt == 0), stop=(kt == nkt - 1),
                            tile_position=(0, 64))
                        nc.tensor.matmul(
                            ctxB_t[32:33, rs0:QW],
                            lhsT=V_sb[kt][:, 2 * pr + 1, DK:DK + 1],
                            rhs=et[:, 1, rs0:QW],
                            start=(kt == 0), stop=(kt == nkt - 1),
                            tile_position=(0, 32))
                    # ctxA rows: V@0:64, sum@64 ; ctxB rows: sum@32, V@64:128
                    nc.vector.tensor_copy(out=cpair[pr][0:DK, qs],
                                          in_=ctxA[0:DK, :])
                    nc.vector.tensor_copy(out=cpair[pr][64:P, qs],
                                          in_=ctxB_t[64:P, :])
                    srow = stg_pool.tile([P, QW], f32, name="srow", tag="sr")
                    nc.vector.tensor_copy(out=srow[64:65, :],
                                          in_=ctxA_t[DK:DK + 1, :])
                    nc.vector.tensor_copy(out=srow[32:33, :],
                                          in_=ctxB_t[32:33, :])
                    nc.sync.dma_start(out=rsums[2 * pr:2 * pr + 1, qs],
                                      in_=srow[64:65, :])
                    nc.sync.dma_start(out=rsums[2 * pr + 1:2 * pr + 2, qs],
                                      in_=srow[32:33, :])

            # ---- orchestration: all of Q first (so the xv load, which
            # shares the xq buffer, can start), then K g0 unblocks pair-0
            # attention; V-projection groups and the g1 K-projection hide
            # under pair-0's exp stream.
            qk_proj_g(xq_sb, wq_sb, 0, QT_sb, 0)
            qk_proj_g(xq_sb, wq_sb, 0, QT_sb, 1)
            qk_proj_g(xk_sb, wk_sb, 2, KT_sb, 0)
            for q in range(NQT):
                att_q(0, q,
                      pre_ctx=(lambda q=q: v_proj(range(4 * q, 4 * q + 4))))
            qk_proj_g(xk_sb, wk_sb, 2, KT_sb, 1)
            for q in range(NQT):
                att_q(1, q)

            # ---- normalize ctx -> ctxN (bf16)
            nc.vector.reciprocal(out=rsums, in_=rsums)
            nc.vector.tensor_copy(out=rsr, in_=rsums)
            for pr in range(2):
                for q in range(NQT):
                    qs = slice(q * QW, (q + 1) * QW)
                    psb = pp_ps.tile([P, QW], f32, name="bcps", tag="pp")
                    nc.tensor.matmul(
                        psb,
                        lhsT=selp_sb[:, pr * P:(pr + 1) * P],
                        rhs=rsr[:, qs],
                        start=True, stop=True)
                    nc.vector.tensor_mul(ctxN[pr][:, qs], cpair[pr][:, qs], psb)

            # ---- O projection -> rs_in (bf16; one DMA per q-block,
            # alternating between the two HWDGE queues)
            for qb in range(NST):
                o_sb = out_pool.tile([P, 2 * QW], bf16, name="o_sb", tag="ob")
                for dh in range(2):
                    ps = pp_ps.tile([P, QW], f32, name="ops", tag="pp")
                    for pr in range(2):
                        nc.tensor.matmul(
                            ps, lhsT=ctxN[pr][:, qb * P:(qb + 1) * P],
                            rhs=wo_sb[pr][:, dh * QW:(dh + 1) * QW],
                            start=(pr == 0), stop=(pr == 1))
                    if dh == 0:
                        nc.vector.tensor_copy(out=o_sb[:, 0:QW], in_=ps)
                    else:
                        nc.scalar.copy(out=o_sb[:, QW:2 * QW], in_=ps)
                eng = nc.sync if qb % 2 == 0 else nc.scalar
                eng.dma_start(out=rs_in[qb * P:(qb + 1) * P, :], in_=o_sb)

        def emit_finish():
            # ---- ReduceScatter over the batch group (bf16)
            nc.gpsimd.collective_compute(
                "ReduceScatter", Alu.add, replica_groups=GROUPS,
                ins=[rs_in.opt()], outs=[rs_out.opt()])

            lng_sb = consts.tile([P, D], f32, name="lng_sb")
            nc.scalar.dma_start(out=lng_sb, in_=lngb)
            lnb_sb = consts.tile([P, D], f32, name="lnb_sb")
            nc.scalar.dma_start(out=lnb_sb, in_=lnbb)

            # ---- residual + LayerNorm on local rows
            for t in range(SQ // P):
                rsl = slice(t * P, (t + 1) * P)
                rs_sb = ln_pool.tile([P, D], bf16, name="rs_sb", tag="lrs")
                nc.sync.dma_start(out=rs_sb, in_=rs_out[rsl, :])
                xr_sb = ln_pool.tile([P, D], f32, name="xr_sb", tag="lr")
                nc.sync.dma_start(out=xr_sb, in_=xres[rsl, :])
                x_sb = ln_pool.tile([P, D], f32, name="x_sb", tag="lx")
                nc.vector.tensor_copy(out=x_sb, in_=rs_sb)
                nc.vector.tensor_add(x_sb, x_sb, xr_sb)
                stats = ln_pool.tile([P, 2, 6], f32, name="stats", tag="lst")
                for sg in range(2):
                    nc.vector.bn_stats(out=stats[:, sg, :],
                                       in_=x_sb[:, sg * QW:(sg + 1) * QW])
                mv = ln_pool.tile([P, 2], f32, name="mv", tag="lmv")
                nc.vector.bn_aggr(out=mv, in_=stats)
                nc.scalar.activation(out=mv[:, 1:2], in_=mv[:, 1:2],
                                     func=Act.Sqrt, bias=eps_sb, scale=1.0)
                nc.vector.reciprocal(out=mv[:, 1:2], in_=mv[:, 1:2])
                nc.vector.tensor_scalar(
                    out=x_sb, in0=x_sb, scalar1=mv[:, 0:1], scalar2=mv[:, 1:2],
                    op0=Alu.subtract, op1=Alu.mult)
                nc.vector.tensor_mul(x_sb, x_sb, lng_sb)
                nc.vector.tensor_add(x_sb, x_sb, lnb_sb)
                nc.sync.dma_start(out=y[rsl, :], in_=x_sb)

        if n_rep == 1:
            emit_body()
        else:
            with tc.For_i(0, n_rep, 1):
                emit_body()
        emit_finish()

    nc.compile()
    _BUILD_CACHE[key] = nc
    return nc


def _make_masks():
    # mask[j][p, half*512 + f] = 1.0 if p + j*128 <= f else 0 (same both halves;
    # halves hold the two heads of a pair for the same k-tile)
    m = np.zeros((4, P, 2 * QW), dtype=np.float32)
    p = np.arange(P)[:, None]
    f = np.arange(QW)[None, :]
    for j in range(4):
        o = j * P
        keep = (p + o <= f)
        m[j][:, 0:QW] = keep
        m[j][:, QW:2 * QW] = keep
    return m.astype(BF)


def _make_selp():
    sp = np.zeros((HL, 2 * P), dtype=np.float32)
    mm = np.arange(P)
    for pr in range(2):
        for k in range(HL):
            sp[k, pr * P:(pr + 1) * P] = (k == 2 * pr + mm // 64)
    return sp


def _prep_in_maps(query, key, value, complexity, wq, bq, wk, bk, wv, bv,
                  wo, bo, ln_g, ln_b, cpen):
    masks2 = _make_masks()
    selp = _make_selp()
    lngb = np.ascontiguousarray(
        np.broadcast_to(np.asarray(ln_g, np.float32)[None, :], (P, D)))
    lnbb = np.ascontiguousarray(
        np.broadcast_to(np.asarray(ln_b, np.float32)[None, :], (P, D)))
    per_batch = []
    for b in range(B):
        xqT = np.ascontiguousarray(query[b].astype(BF).T)
        xkT = np.ascontiguousarray(key[b].astype(BF).T)
        xvT = np.ascontiguousarray(value[b].astype(BF).T)
        e = np.exp(-float(cpen) * complexity[b].astype(np.float64)).astype(np.float32)
        enb_l = np.ascontiguousarray(e.reshape(NST, P).T)
        per_batch.append((xqT, xkT, xvT, enb_l))
    in_maps = []
    for c in range(8):
        b, r = c // 4, c % 4
        hs = HD * r
        xqT, xkT, xvT, enb_l = per_batch[b]
        bqc = bq[hs:hs + HD].astype(np.float32).reshape(2, P).T  # [P, 2]
        bkc = bk[hs:hs + HD].astype(np.float32).reshape(2, P).T
        bqk_l = np.ascontiguousarray(
            np.concatenate([bqc, bkc], axis=1))  # [P,4]: q_g0,q_g1,k_g0,k_g1
        in_maps.append({
            "xqT": xqT, "xkT": xkT, "xvT": xvT,
            "wqT": np.ascontiguousarray(wq[hs:hs + HD, :].T).astype(BF),
            "wkT": np.ascontiguousarray(wk[hs:hs + HD, :].T).astype(BF),
            "wvT": np.ascontiguousarray(wv[hs:hs + HD, :].T).astype(BF),
            "woT": np.ascontiguousarray(wo[:, hs:hs + HD].T).astype(BF),
            "bqk": bqk_l,
            "bvm": bv[hs:hs + HD].astype(BF)[None, :],
            "enb": enb_l,
            "masks2": masks2,
            "selp": selp,
            "xres": (query[b][SQ * r:SQ * (r + 1)].astype(np.float32)
                     + bo.astype(np.float32)[None, :]),
            "lngb": lngb, "lnbb": lnbb,
        })
    return in_maps


def _numpy_fallback(query, key, value, complexity, mask, wq, bq, wk, bk,
                    wv, bv, wo, bo, ln_g, ln_b, cpen):
    import math
    out = np.zeros((B, S, D), np.float32)
    for b in range(B):
        Q = query[b] @ wq.T + bq
        K = key[b] @ wk.T + bk
        V = value[b] @ wv.T + bv
        Qh = Q.reshape(S, H, DK).transpose(1, 0, 2)
        Kh = K.reshape(S, H, DK).transpose(1, 0, 2)
        Vh = V.reshape(S, H, DK).transpose(1, 0, 2)
        ctx = np.zeros((H, S, DK), np.float32)
        m = mask[b, 0]
        for h in range(H):
            sc = Qh[h] @ Kh[h].T / math.sqrt(DK) - cpen * complexity[b][None, :]
            sc = np.where(m, sc, -1e9)
            sc = sc - sc.max(-1, keepdims=True)
            e = np.exp(sc)
            a = e / e.sum(-1, keepdims=True)
            ctx[h] = a @ Vh[h]
        x = ctx.transpose(1, 0, 2).reshape(S, D) @ wo.T + bo + query[b]
        mu = x.mean(-1, keepdims=True)
        var = ((x - mu) ** 2).mean(-1, keepdims=True)
        out[b] = (x - mu) / np.sqrt(var + EPS) * ln_g + ln_b
    return out


_TRIL = None


def kernel(query, key, value, complexity, mask, wq, bq, wk, bk, wv, bv,
           wo, bo, ln_g, ln_b, cpen, **_unused):
    query = np.asarray(query, dtype=np.float32)
    key = np.asarray(key, dtype=np.float32)
    value = np.asarray(value, dtype=np.float32)
    complexity = np.asarray(complexity, dtype=np.float32)
    mask = np.asarray(mask)
    args = dict(query=query, key=key, value=value, complexity=complexity,
                wq=np.asarray(wq), bq=np.asarray(bq), wk=np.asarray(wk),
                bk=np.asarray(bk), wv=np.asarray(wv), bv=np.asarray(bv),
                wo=np.asarray(wo), bo=np.asarray(bo),
                ln_g=np.asarray(ln_g), ln_b=np.asarray(ln_b),
                cpen=float(np.asarray(cpen)))
    global _TRIL
    if _TRIL is None:
        _TRIL = np.tril(np.ones((S, S), bool))
    if not all(np.array_equal(mask[b, 0], _TRIL) for b in range(B)):
        # non-causal mask: fall back to a generic host implementation
        return _numpy_fallback(mask=mask, **args)

    ex = _get_exec()
    in_maps = _prep_in_maps(**args)
    outs = ex.run(ex.stage(in_maps))
    res = ex.results(outs)
    out = np.empty((B, S, D), np.float32)
    for c in range(8):
        b, r = c // 4, c % 4
        out[b, SQ * r:SQ * (r + 1)] = res[c]["y"]
    return out


def _get_exec():
    if "ex" not in _BUILD_CACHE:
        _BUILD_CACHE["ex"] = _Exec(_build())
    return _BUILD_CACHE["ex"]


def _input_args(inputs):
    return dict(query=np.asarray(inputs["query"], np.float32),
                key=np.asarray(inputs["key"], np.float32),
                value=np.asarray(inputs["value"], np.float32),
                complexity=np.asarray(inputs["complexity"], np.float32),
                wq=np.asarray(inputs["wq"]), bq=np.asarray(inputs["bq"]),
                wk=np.asarray(inputs["wk"]), bk=np.asarray(inputs["bk"]),
                wv=np.asarray(inputs["wv"]), bv=np.asarray(inputs["bv"]),
                wo=np.asarray(inputs["wo"]), bo=np.asarray(inputs["bo"]),
                ln_g=np.asarray(inputs["ln_g"]), ln_b=np.asarray(inputs["ln_b"]),
                cpen=float(np.asarray(inputs["cpen"])))


# revision 107
# speedup vs baseline: 1.4271x; 1.1028x over previous
"""Trainium2 Bass kernel: ComplexityAwareAttention (B=2, S=2048, D=1024, H=16).

Sharding: 8 cores = 2 batches x 4 head-groups (4 heads each).
Per core: QKV projections (bf16, head-slice, inputs pre-transposed on host),
flash-style causal attention with no-max softmax (scores bounded ~+-3.4),
complexity bias folded into V rows as exp(-cpen*c_k), partial O-projection,
bf16 ReduceScatter over the 4-core batch group, residual + LayerNorm on the
local quarter of rows.
"""

import numpy as np
import ml_dtypes
from contextlib import ExitStack

import concourse.bass as bass
import concourse.bacc as bacc
import concourse.tile as tile
from concourse import mybir
from concourse.bass_utils import run_bass_kernel_spmd


class _Exec:
    """Cached jit executor mirroring bass2jax.run_bass_via_pjrt (axon path),
    so repeat kernel() calls skip retracing and host->device re-staging of
    unchanged inputs can be controlled by the caller."""

    def __init__(self, nc, n_cores=8):
        import jax
        from jax.sharding import Mesh, PartitionSpec
        from jax.experimental.shard_map import shard_map
        from concourse import bass2jax
        from concourse import mybir as mb

        bass2jax.install_neuronx_cc_hook()
        assert nc.dbg_addr is None
        partition_name = (nc.partition_id_tensor.name
                          if nc.partition_id_tensor else None)
        in_names, out_names, out_avals = [], [], []
        for alloc in nc.m.functions[0].allocations:
            if not isinstance(alloc, mb.MemoryLocationSet):
                continue
            name = alloc.memorylocations[0].name
            if alloc.kind == "ExternalInput":
                if name != partition_name:
                    in_names.append(name)
            elif alloc.kind == "ExternalOutput":
                shape = tuple(alloc.tensor_shape)
                dtype = mb.dt.np(alloc.dtype)
                out_names.append(name)
                out_avals.append(jax.core.ShapedArray(shape, dtype))
        self.nc = nc
        self.in_names = in_names
        self.out_names = out_names
        self.out_avals = out_avals
        self.n_cores = n_cores
        n_params = len(in_names)
        n_outs = len(out_names)
        donate = tuple(range(n_params, n_params + n_outs))
        all_names = in_names + out_names
        if partition_name is not None:
            all_names = all_names + [partition_name]

        def _body(*args):
            operands = list(args)
            if partition_name is not None:
                operands.append(bass2jax.partition_id_tensor())
            return tuple(bass2jax._bass_exec_p.bind(
                *operands,
                out_avals=tuple(out_avals),
                in_names=tuple(all_names),
                out_names=tuple(out_names),
                lowering_input_output_aliases=(),
                sim_require_finite=True,
                sim_require_nnan=True,
                nc=nc,
            ))

        devices = jax.devices()[:n_cores]
        self.mesh = Mesh(np.asarray(devices), ("core",))
        in_specs = (PartitionSpec("core"),) * (n_params + n_outs)
        out_specs = (PartitionSpec("core"),) * n_outs
        self.sharded = jax.jit(
            shard_map(_body, mesh=self.mesh, in_specs=in_specs,
                      out_specs=out_specs, check_rep=False),
            donate_argnums=donate, keep_unused=True)
        self._jax = jax

    def stage(self, in_maps):
        """Concatenate per-core inputs and move to devices; returns arg list."""
        import jax
        from jax.sharding import NamedSharding, PartitionSpec
        sh = NamedSharding(self.mesh, PartitionSpec("core"))
        args = []
        for name in self.in_names:
            cat = np.concatenate([np.asarray(m[name]) for m in in_maps], axis=0)
            args.append(jax.device_put(cat, sh))
        return args

    def zeros(self):
        import jax
        from jax.sharding import NamedSharding, PartitionSpec
        sh = NamedSharding(self.mesh, PartitionSpec("core"))
        return [jax.device_put(
                    np.zeros((self.n_cores * a.shape[0], *a.shape[1:]), a.dtype), sh)
                for a in self.out_avals]

    def run(self, staged_args, zeros=None):
        if zeros is None:
            zeros = self.zeros()
        outs = self.sharded(*staged_args, *zeros)
        self._jax.block_until_ready(outs)
        return outs

    def make_chain(self, n):
        """Jitted fn running the program n times back-to-back on device;
        outputs of call i feed the (ignored, fully-overwritten) output
        buffers of call i+1, forcing serial execution."""
        import jax
        from jax.sharding import PartitionSpec
        from jax.experimental.shard_map import shard_map
        from concourse import bass2jax

        partition_name = (self.nc.partition_id_tensor.name
                          if self.nc.partition_id_tensor else None)
        all_names = self.in_names + self.out_names
        if partition_name is not None:
            all_names = all_names + [partition_name]
        out_avals = self.out_avals
        nc = self.nc

        def _body(*args):
            n_params = len(self.in_names)
            ins = list(args[:n_params])
            z = list(args[n_params:])
            for _ in range(n):
                operands = ins + z
                if partition_name is not None:
                    operands.append(bass2jax.partition_id_tensor())
                outs = bass2jax._bass_exec_p.bind(
                    *operands,
                    out_avals=tuple(out_avals),
                    in_names=tuple(all_names),
                    out_names=tuple(self.out_names),
                    lowering_input_output_aliases=(),
                    sim_require_finite=True,
                    sim_require_nnan=True,
                    nc=nc,
                )
                z = list(outs)
            return tuple(z)

        n_io = len(self.in_names) + len(self.out_names)
        in_specs = (PartitionSpec("core"),) * n_io
        out_specs = (PartitionSpec("core"),) * len(self.out_names)
        return jax.jit(
            shard_map(_body, mesh=self.mesh, in_specs=in_specs,
                      out_specs=out_specs, check_rep=False),
            keep_unused=True)

    def results(self, outs):
        per_core = []
        for c in range(self.n_cores):
            d = {}
            for i, name in enumerate(self.out_names):
                a = self.out_avals[i]
                d[name] = np.asarray(outs[i]).reshape(
                    self.n_cores, *a.shape)[c]
            per_core.append(d)
        return per_core

# ---- problem constants (hardcoded per harness contract)
B, S, D, H = 2, 2048, 1024, 16
DK = D // H                      # 64
EPS = 1e-5
P = 128
HL = 4                           # heads per core
HD = HL * DK                     # 256 local head dims
SQ = S // 4                      # 512 output rows per core
NDT = D // P                     # 8 d-tiles
NQT = 4                          # q tiles of 512
QW = 512
NST = S // P                     # 16 s/k tiles
GROUPS = [[0, 1, 2, 3], [4, 5, 6, 7]]

f32 = mybir.dt.float32
bf16 = mybir.dt.bfloat16
f32r = mybir.dt.float32r
BF = ml_dtypes.bfloat16

Alu = mybir.AluOpType
Act = mybir.ActivationFunctionType

_BUILD_CACHE = {}

_ET_BUFS = 6  # exp-tile ring depth (absorbs ACT-vs-PE skew)


def _build(n_rep=1):
    key = ("nc", n_rep, _ET_BUFS)
    if key in _BUILD_CACHE:
        return _BUILD_CACHE[key]

    nc = bacc.Bacc("TRN2", target_bir_lowering=False, debug=False,
                   enable_asserts=False, num_devices=8)

    # ---- DRAM I/O (x inputs pre-transposed on host: [D, S])
    xqT_t = nc.dram_tensor("xqT", [D, S], bf16, kind="ExternalInput")
    xkT_t = nc.dram_tensor("xkT", [D, S], bf16, kind="ExternalInput")
    xvT_t = nc.dram_tensor("xvT", [D, S], bf16, kind="ExternalInput")
    wqT = nc.dram_tensor("wqT", [D, HD], bf16, kind="ExternalInput").ap()
    wkT = nc.dram_tensor("wkT", [D, HD], bf16, kind="ExternalInput").ap()
    wvT = nc.dram_tensor("wvT", [D, HD], bf16, kind="ExternalInput").ap()
    woT = nc.dram_tensor("woT", [HD, D], bf16, kind="ExternalInput").ap()
    bqk = nc.dram_tensor("bqk", [P, 4], f32, kind="ExternalInput").ap()
    bvm = nc.dram_tensor("bvm", [1, HD], bf16, kind="ExternalInput").ap()
    enb = nc.dram_tensor("enb", [P, NST], f32, kind="ExternalInput").ap()
    masks2 = nc.dram_tensor("masks2", [4, P, 2 * QW], bf16, kind="ExternalInput").ap()
    selp = nc.dram_tensor("selp", [HL, 2 * P], f32r, kind="ExternalInput").ap()
    xres = nc.dram_tensor("xres", [SQ, D], f32, kind="ExternalInput").ap()
    lngb = nc.dram_tensor("lngb", [P, D], f32, kind="ExternalInput").ap()
    lnbb = nc.dram_tensor("lnbb", [P, D], f32, kind="ExternalInput").ap()
    y = nc.dram_tensor("y", [SQ, D], f32, kind="ExternalOutput").ap()

    rs_in = nc.dram_tensor("rs_in", [S, D], bf16).ap()
    rs_out = nc.dram_tensor("rs_out", [SQ, D], bf16).ap()

    with tile.TileContext(nc) as tc, ExitStack() as ctx:
        consts = ctx.enter_context(tc.tile_pool(name="consts", bufs=1))
        xt_pool = ctx.enter_context(tc.tile_pool(name="xt", bufs=1))
        et_pool = ctx.enter_context(tc.tile_pool(name="et", bufs=_ET_BUFS))
        stg_pool = ctx.enter_context(tc.tile_pool(name="stg", bufs=2))
        out_pool = ctx.enter_context(tc.tile_pool(name="outp", bufs=2))
        ln_pool = ctx.enter_context(tc.tile_pool(name="ln", bufs=2))
        pp_ps = ctx.enter_context(tc.tile_pool(name="pp_ps", bufs=2, space="PSUM"))
        s_ps = ctx.enter_context(tc.tile_pool(name="s_ps", bufs=2, space="PSUM"))
        c_ps = ctx.enter_context(tc.tile_pool(name="c_ps", bufs=1, space="PSUM"))

        # ---- true constants (outside the timing loop)
        ones_sb = consts.tile([1, P], bf16, name="ones_sb")
        nc.vector.memset(ones_sb, 1.0)
        eps_sb = consts.tile([P, 1], f32, name="eps_sb")
        nc.vector.memset(eps_sb, EPS)
        mask_sb = []
        for j in range(4):
            t = consts.tile([P, 2 * QW], bf16, name=f"mask{j}")
            nc.sync.dma_start(out=t, in_=masks2[j])
            mask_sb.append(t)
        selp_sb = consts.tile([HL, 2 * P], f32r, name="selp_sb")
        nc.sync.dma_start(out=selp_sb, in_=selp)

        # persistent activation tiles
        QT_sb = [consts.tile([P, S], bf16, name=f"QT{g}") for g in range(2)]
        KT_sb = [consts.tile([P, S], bf16, name=f"KT{g}") for g in range(2)]
        # V tiles: [V(64), enb] per head
        V_sb = [consts.tile([P, HL, DK + 1], bf16, name=f"V{st}") for st in range(NST)]
        cpair = [consts.tile([P, S], f32, name=f"cp{pr}") for pr in range(2)]
        ctxN = [consts.tile([P, S], bf16, name=f"cn{pr}") for pr in range(2)]
        rsums = consts.tile([HL, S], f32, name="rsums")
        rsr = consts.tile([HL, S], f32r, name="rsr")
        # weight tiles (persistent slots; reloaded per iteration in-loop)
        wq_sb = [consts.tile([P, HD], bf16, name=f"wq{d}") for d in range(NDT)]
        wk_sb = [consts.tile([P, HD], bf16, name=f"wk{d}") for d in range(NDT)]
        wv_sb = [consts.tile([P, HD], bf16, name=f"wv{d}") for d in range(NDT)]
        wo_sb = [consts.tile([P, D], bf16, name=f"wo{j}") for j in range(2)]
        bqk_sb = consts.tile([P, 4], f32, name="bqk_sb")
        bvm_sb = consts.tile([1, HD], bf16, name="bvm_sb")
        enb_sb = consts.tile([P, NST], f32, name="enb_sb")

        def load_x(dst, src_t, nchunk, eng):
            """DMA [D, S] dram -> [128, NDT*S] sbuf in nchunk pieces."""
            nd = NDT // nchunk
            for c in range(nchunk):
                d0 = c * nd
                dst3 = dst[:, d0 * S:(d0 + nd) * S].rearrange(
                    "p (d s) -> p d s", d=nd)
                src3 = bass.AP(tensor=src_t, offset=d0 * P * S,
                               ap=[[S, P], [P * S, nd], [1, S]])
                eng.dma_start(out=dst3, in_=src3)

        def emit_body():
            # ---- input/weight loads.
            # scalar (ACT) HWDGE queue: wq, bqk, wk, xkT, wv, bvm, enb
            #   (everything gating QK proj + V proj; done before exp stream)
            # sync (SP) HWDGE queue: xqT, xvT, wo (then rsum/rs_in writes)
            for d in range(NDT):
                nc.scalar.dma_start(out=wq_sb[d], in_=wqT[d * P:(d + 1) * P, :])
            nc.scalar.dma_start(out=bqk_sb, in_=bqk)
            for d in range(NDT):
                nc.scalar.dma_start(out=wk_sb[d], in_=wkT[d * P:(d + 1) * P, :])

            xq_sb = xt_pool.tile([P, NDT * S], bf16, name="xq_sb", tag="xq")
            load_x(xq_sb, xqT_t, 4, nc.sync)
            xk_sb = xt_pool.tile([P, NDT * S], bf16, name="xk_sb", tag="xk")
            load_x(xk_sb, xkT_t, 2, nc.scalar)

            for d in range(NDT):
                nc.scalar.dma_start(out=wv_sb[d], in_=wvT[d * P:(d + 1) * P, :])
            nc.scalar.dma_start(out=bvm_sb, in_=bvm)
            nc.scalar.dma_start(out=enb_sb, in_=enb)

            xv_sb = xt_pool.tile([P, NDT * S], bf16, name="xv_sb", tag="xq")
            load_x(xv_sb, xvT_t, 2, nc.sync)
            for j in range(2):
                nc.sync.dma_start(out=wo_sb[j], in_=woT[j * P:(j + 1) * P, :])

            # ---- Q/K projections -> QT/KT [2][128(2 heads x 64), S]
            def qk_proj_g(x_sb, w_sb, bbase, out_sb, g):
                for q in range(NQT):
                    ps = pp_ps.tile([P, QW], f32, name="pp", tag="pp")
                    for d in range(NDT):
                        nc.tensor.matmul(
                            ps, lhsT=w_sb[d][:, g * P:(g + 1) * P],
                            rhs=x_sb[:, d * S + q * QW:d * S + (q + 1) * QW],
                            start=(d == 0), stop=(d == NDT - 1))
                    nc.vector.tensor_scalar(
                        out=out_sb[g][:, q * QW:(q + 1) * QW], in0=ps,
                        scalar1=bqk_sb[:, bbase + g:bbase + g + 1],
                        scalar2=None, op0=Alu.add)

            # ---- V projection (st tile group)
            def v_proj(sts):
                for st in sts:
                    ps = pp_ps.tile([P, QW], f32, name="ppv", tag="pp")
                    psv = ps[:, 0:HD]
                    nc.tensor.matmul(psv, lhsT=ones_sb[0:1, 0:P], rhs=bvm_sb,
                                     start=True, stop=False)
                    for d in range(NDT):
                        nc.tensor.matmul(
                            psv, lhsT=xv_sb[:, d * S + st * P:d * S + (st + 1) * P],
                            rhs=wv_sb[d],
                            start=False, stop=(d == NDT - 1))
                    psr = psv.rearrange("p (h c) -> p h c", h=HL)
                    nc.vector.tensor_scalar(
                        out=V_sb[st][:, :, 0:DK], in0=psr,
                        scalar1=enb_sb[:, st:st + 1], scalar2=None, op0=Alu.mult)
                    nc.vector.tensor_copy(
                        out=V_sb[st][:, :, DK:DK + 1],
                        in_=enb_sb[:, st:st + 1].to_broadcast([P, HL, 1]))

            # ---- attention for one (pair, q-tile); pre_ctx() emits work
            # (e.g. the V projection group) between the scores/exp stream
            # and the ctx matmuls that consume V
            def att_q(pr, q, pre_ctx=None):
                    g = pr
                    nkt = 4 * (q + 1)
                    qs = slice(q * QW, (q + 1) * QW)
                    ctxA_t = c_ps.tile([P, QW], f32, name="ctxA", tag="cpA")
                    ctxB_t = c_ps.tile([P, QW], f32, name="ctxB", tag="cpB")
                    ctxA = ctxA_t[0:DK + 1, :]
                    ets = []
                    for kt in range(nkt):
                        j = kt - 4 * q  # >=0 on diagonal group
                        rs0 = 128 * j if j >= 0 else 0
                        # scoresT for both heads of the pair: 2 row-tiled
                        # [K=64, M=128, N<=512] matmuls (PE row halves) —
                        # half the streaming cycles of the 4-quadrant form
                        ps = s_ps.tile([P, 2 * QW], f32, name="sc", tag="sp")
                        for hh in range(2):
                            ho = hh * 64
                            nc.tensor.matmul(
                                ps[:, hh * QW + rs0:(hh + 1) * QW],
                                lhsT=KT_sb[g][ho:ho + 64,
                                              kt * P:(kt + 1) * P],
                                rhs=QT_sb[g][ho:ho + 64,
                                             q * QW + rs0:(q + 1) * QW],
                                start=True, stop=True,
                                tile_position=(ho, 0))
                        et = et_pool.tile([P, 2, QW], bf16, name="et", tag="et")
                        ps3 = ps.rearrange("p (h f) -> p h f", h=2)
                        nc.scalar.activation(out=et[:, :, rs0:QW],
                                             in_=ps3[:, :, rs0:QW],
                                             func=Act.Exp, scale=0.125)
                        if j >= 0:
                            nc.vector.tensor_mul(
                                et[:, :, rs0:QW],
                                et[:, :, rs0:QW],
                                mask_sb[j].rearrange(
                                    "p (h f) -> p h f", h=2)[:, :, rs0:QW])
                        ets.append((et, rs0))
                    if pre_ctx is not None:
                        pre_ctx()
                    for kt, (et, rs0) in enumerate(ets):
                        # both heads M=65 at base 0 (V rows + enb-sum row);
                        # odd head's rows reach cpair[64:128] via one staged
                        # partition-shift DMA per q-tile below
                        nc.tensor.matmul(
                            ctxA[:, rs0:QW], lhsT=V_sb[kt][:, 2 * pr, :],
                            rhs=et[:, 0, rs0:QW],
                            start=(kt == 0), stop=(kt == nkt - 1))
                        nc.tensor.matmul(
                            ctxB_t[0:DK + 1, rs0:QW],
                            lhsT=V_sb[kt][:, 2 * pr + 1, :],
                            rhs=et[:, 1, rs0:QW],
                            start=(kt == 0), stop=(kt == nkt - 1))
                    # ctxA/B rows: V@0:64, sum@64
                    nc.vector.tensor_copy(out=cpair[pr][0:DK, qs],
                                          in_=ctxA[0:DK, :])
                    stg2 = stg_pool.tile([P, QW], f32, name="stg2", tag="sg2")
                    nc.vector.tensor_copy(out=stg2[0:DK + 1, :],
                                          in_=ctxB_t[0:DK + 1, :])
                    nc.sync.dma_start(out=cpair[pr][64:P, qs],
                                      in_=stg2[0:DK, :])
                    srow = stg_pool.tile([P, QW], f32, name="srow", tag="sr")
                    nc.vector.tensor_copy(out=srow[64:65, :],
                                          in_=ctxA_t[DK:DK + 1, :])
                    nc.sync.dma_start(out=rsums[2 * pr:2 * pr + 1, qs],
                                      in_=srow[64:65, :])
                    nc.sync.dma_start(out=rsums[2 * pr + 1:2 * pr + 2, qs],
                                      in_=stg2[DK:DK + 1, :])

            # ---- orchestration: all of Q first (so the xv load, which
            # shares the xq buffer, can start), then K g0 unblocks pair-0
            # attention; V-projection groups and the g1 K-projection hide
            # under pair-0's exp stream.
            qk_proj_g(xq_sb, wq_sb, 0, QT_sb, 0)
            qk_proj_g(xq_sb, wq_sb, 0, QT_sb, 1)
            qk_proj_g(xk_sb, wk_sb, 2, KT_sb, 0)
            for q in range(NQT):
                att_q(0, q,
                      pre_ctx=(lambda q=q: v_proj(range(4 * q, 4 * q + 4))))
            qk_proj_g(xk_sb, wk_sb, 2, KT_sb, 1)
            for q in range(NQT):
                att_q(1, q)

            # ---- normalize ctx -> ctxN (bf16)
            nc.vector.reciprocal(out=rsums, in_=rsums)
            nc.vector.tensor_copy(out=rsr, in_=rsums)
            for pr in range(2):
                for q in range(NQT):
                    qs = slice(q * QW, (q + 1) * QW)
                    psb = pp_ps.tile([P, QW], f32, name="bcps", tag="pp")
                    nc.tensor.matmul(
                        psb,
                        lhsT=selp_sb[:, pr * P:(pr + 1) * P],
                        rhs=rsr[:, qs],
                        start=True, stop=True)
                    nc.vector.tensor_mul(ctxN[pr][:, qs], cpair[pr][:, qs], psb)

            # ---- O projection -> rs_in (bf16; one DMA per q-block,
            # alternating between the two HWDGE queues)
            for qb in range(NST):
                o_sb = out_pool.tile([P, 2 * QW], bf16, name="o_sb", tag="ob")
                for dh in range(2):
                    ps = pp_ps.tile([P, QW], f32, name="ops", tag="pp")
                    for pr in range(2):
                        nc.tensor.matmul(
                            ps, lhsT=ctxN[pr][:, qb * P:(qb + 1) * P],
                            rhs=wo_sb[pr][:, dh * QW:(dh + 1) * QW],
                            start=(pr == 0), stop=(pr == 1))
                    if dh == 0:
                        nc.vector.tensor_copy(out=o_sb[:, 0:QW], in_=ps)
                    else:
                        nc.scalar.copy(out=o_sb[:, QW:2 * QW], in_=ps)
                eng = nc.sync if qb % 2 == 0 else nc.scalar
                eng.dma_start(out=rs_in[qb * P:(qb + 1) * P, :], in_=o_sb)

        def emit_finish():
            # ---- ReduceScatter over the batch group (bf16)
            nc.gpsimd.collective_compute(
                "ReduceScatter", Alu.add, replica_groups=GROUPS,
                ins=[rs_in.opt()], outs=[rs_out.opt()])

            lng_sb = consts.tile([P, D], f32, name="lng_sb")
            nc.scalar.dma_start(out=lng_sb, in_=lngb)
            lnb_sb = consts.tile([P, D], f32, name="lnb_sb")
            nc.scalar.dma_start(out=lnb_sb, in_=lnbb)

            # ---- residual + LayerNorm on local rows
            for t in range(SQ // P):
                rsl = slice(t * P, (t + 1) * P)
                rs_sb = ln_pool.tile([P, D], bf16, name="rs_sb", tag="lrs")
                nc.sync.dma_start(out=rs_sb, in_=rs_out[rsl, :])
                xr_sb = ln_pool.tile([P, D], f32, name="xr_sb", tag="lr")
                nc.sync.dma_start(out=xr_sb, in_=xres[rsl, :])
                x_sb = ln_pool.tile([P, D], f32, name="x_sb", tag="lx")
                nc.vector.tensor_copy(out=x_sb, in_=rs_sb)
                nc.vector.tensor_add(x_sb, x_sb, xr_sb)
                stats = ln_pool.tile([P, 2, 6], f32, name="stats", tag="lst")
                for sg in range(2):
                    nc.vector.bn_stats(out=stats[:, sg, :],
                                       in_=x_sb[:, sg * QW:(sg + 1) * QW])
                mv = ln_pool.tile([P, 2], f32, name="mv", tag="lmv")
                nc.vector.bn_aggr(out=mv, in_=stats)
                nc.scalar.activation(out=mv[:, 1:2], in_=mv[:, 1:2],
                                     func=Act.Sqrt, bias=eps_sb, scale=1.0)
                nc.vector.reciprocal(out=mv[:, 1:2], in_=mv[:, 1:2])
                nc.vector.tensor_scalar(
                    out=x_sb, in0=x_sb, scalar1=mv[:, 0:1], scalar2=mv[:, 1:2],
                    op0=Alu.subtract, op1=Alu.mult)
                nc.vector.tensor_mul(x_sb, x_sb, lng_sb)
                nc.vector.tensor_add(x_sb, x_sb, lnb_sb)
                nc.sync.dma_start(out=y[rsl, :], in_=x_sb)

        if n_rep == 1:
            emit_body()
        else:
            with tc.For_i(0, n_rep, 1):
                emit_body()
        emit_finish()

    nc.compile()
    _BUILD_CACHE[key] = nc
    return nc


def _make_masks():
    # mask[j][p, half*512 + f] = 1.0 if p + j*128 <= f else 0 (same both halves;
    # halves hold the two heads of a pair for the same k-tile)
    m = np.zeros((4, P, 2 * QW), dtype=np.float32)
    p = np.arange(P)[:, None]
    f = np.arange(QW)[None, :]
    for j in range(4):
        o = j * P
        keep = (p + o <= f)
        m[j][:, 0:QW] = keep
        m[j][:, QW:2 * QW] = keep
    return m.astype(BF)


def _make_selp():
    sp = np.zeros((HL, 2 * P), dtype=np.float32)
    mm = np.arange(P)
    for pr in range(2):
        for k in range(HL):
            sp[k, pr * P:(pr + 1) * P] = (k == 2 * pr + mm // 64)
    return sp


def _prep_in_maps(query, key, value, complexity, wq, bq, wk, bk, wv, bv,
                  wo, bo, ln_g, ln_b, cpen):
    masks2 = _make_masks()
    selp = _make_selp()
    lngb = np.ascontiguousarray(
        np.broadcast_to(np.asarray(ln_g, np.float32)[None, :], (P, D)))
    lnbb = np.ascontiguousarray(
        np.broadcast_to(np.asarray(ln_b, np.float32)[None, :], (P, D)))
    per_batch = []
    for b in range(B):
        xqT = np.ascontiguousarray(query[b].astype(BF).T)
        xkT = np.ascontiguousarray(key[b].astype(BF).T)
        xvT = np.ascontiguousarray(value[b].astype(BF).T)
        e = np.exp(-float(cpen) * complexity[b].astype(np.float64)).astype(np.float32)
        enb_l = np.ascontiguousarray(e.reshape(NST, P).T)
        per_batch.append((xqT, xkT, xvT, enb_l))
    in_maps = []
    for c in range(8):
        b, r = c // 4, c % 4
        hs = HD * r
        xqT, xkT, xvT, enb_l = per_batch[b]
        bqc = bq[hs:hs + HD].astype(np.float32).reshape(2, P).T  # [P, 2]
        bkc = bk[hs:hs + HD].astype(np.float32).reshape(2, P).T
        bqk_l = np.ascontiguousarray(
            np.concatenate([bqc, bkc], axis=1))  # [P,4]: q_g0,q_g1,k_g0,k_g1
        in_maps.append({
            "xqT": xqT, "xkT": xkT, "xvT": xvT,
            "wqT": np.ascontiguousarray(wq[hs:hs + HD, :].T).astype(BF),
            "wkT": np.ascontiguousarray(wk[hs:hs + HD, :].T).astype(BF),
            "wvT": np.ascontiguousarray(wv[hs:hs + HD, :].T).astype(BF),
            "woT": np.ascontiguousarray(wo[:, hs:hs + HD].T).astype(BF),
            "bqk": bqk_l,
            "bvm": bv[hs:hs + HD].astype(BF)[None, :],
            "enb": enb_l,
            "masks2": masks2,
            "selp": selp,
            "xres": (query[b][SQ * r:SQ * (r + 1)].astype(np.float32)
                     + bo.astype(np.float32)[None, :]),
            "lngb": lngb, "lnbb": lnbb,
        })
    return in_maps


def _numpy_fallback(query, key, value, complexity, mask, wq, bq, wk, bk,
                    wv, bv, wo, bo, ln_g, ln_b, cpen):
    import math
    out = np.zeros((B, S, D), np.float32)
    for b in range(B):
        Q = query[b] @ wq.T + bq
        K = key[b] @ wk.T + bk
        V = value[b] @ wv.T + bv
        Qh = Q.reshape(S, H, DK).transpose(1, 0, 2)
        Kh = K.reshape(S, H, DK).transpose(1, 0, 2)
        Vh = V.reshape(S, H, DK).transpose(1, 0, 2)
        ctx = np.zeros((H, S, DK), np.float32)
        m = mask[b, 0]
        for h in range(H):
            sc = Qh[h] @ Kh[h].T / math.sqrt(DK) - cpen * complexity[b][None, :]
            sc = np.where(m, sc, -1e9)
            sc = sc - sc.max(-1, keepdims=True)
            e = np.exp(sc)
            a = e / e.sum(-1, keepdims=True)
            ctx[h] = a @ Vh[h]
        x = ctx.transpose(1, 0, 2).reshape(S, D) @ wo.T + bo + query[b]
        mu = x.mean(-1, keepdims=True)
        var = ((x - mu) ** 2).mean(-1, keepdims=True)
        out[b] = (x - mu) / np.sqrt(var + EPS) * ln_g + ln_b
    return out


_TRIL = None


def kernel(query, key, value, complexity, mask, wq, bq, wk, bk, wv, bv,
           wo, bo, ln_g, ln_b, cpen, **_unused):
    query = np.asarray(query, dtype=np.float32)
    key = np.asarray(key, dtype=np.float32)
    value = np.asarray(value, dtype=np.float32)
    complexity = np.asarray(complexity, dtype=np.float32)
    mask = np.asarray(mask)
    args = dict(query=query, key=key, value=value, complexity=complexity,
                wq=np.asarray(wq), bq=np.asarray(bq), wk=np.asarray(wk),
                bk=np.asarray(bk), wv=np.asarray(wv), bv=np.asarray(bv),
                wo=np.asarray(wo), bo=np.asarray(bo),
                ln_g=np.asarray(ln_g), ln_b=np.asarray(ln_b),
                cpen=float(np.asarray(cpen)))
    global _TRIL
    if _TRIL is None:
        _TRIL = np.tril(np.ones((S, S), bool))
    if not all(np.array_equal(mask[b, 0], _TRIL) for b in range(B)):
        # non-causal mask: fall back to a generic host implementation
        return _numpy_fallback(mask=mask, **args)

    ex = _get_exec()
    in_maps = _prep_in_maps(**args)
    outs = ex.run(ex.stage(in_maps))
    res = ex.results(outs)
    out = np.empty((B, S, D), np.float32)
    for c in range(8):
        b, r = c // 4, c % 4
        out[b, SQ * r:SQ * (r + 1)] = res[c]["y"]
    return out


def _get_exec():
    if "ex" not in _BUILD_CACHE:
        _BUILD_CACHE["ex"] = _Exec(_build())
    return _BUILD_CACHE["ex"]


def _input_args(inputs):
    return dict(query=np.asarray(inputs["query"], np.float32),
                key=np.asarray(inputs["key"], np.float32),
                value=np.asarray(inputs["value"], np.float32),
                complexity=np.asarray(inputs["complexity"], np.float32),
                wq=np.asarray(inputs["wq"]), bq=np.asarray(inputs["bq"]),
                wk=np.asarray(inputs["wk"]), bk=np.asarray(inputs["bk"]),
                wv=np.asarray(inputs["wv"]), bv=np.asarray(inputs["bv"]),
                wo=np.asarray(inputs["wo"]), bo=np.asarray(inputs["bo"]),
                ln_g=np.asarray(inputs["ln_g"]), ln_b=np.asarray(inputs["ln_b"]),
                cpen=float(np.asarray(inputs["cpen"])))
